# revision 15
# baseline (speedup 1.0000x reference)
"""GAT message-passing kernel for Trainium2, 8 NeuronCores, dst-partitioned.

v3 (bf16, paired windows, shipped transposed one-hot, p-major table):
 - Fold attention vectors into the linear weights on host (tiny matmuls):
   a_src = x @ u_src.T, a_dst = x @ u_dst.T, a_edge = edge_attr @ v.T.
 - Softmax over incoming edges is computed WITHOUT max-subtraction (logits
   are bounded so exp cannot overflow; softmax is shift-invariant) so only
   segment-SUMS are needed, which map onto TensorE one-hot matmuls.
 - Host packs destination nodes into 128-slot windows (slot 127 of every
   window is a trash slot that absorbs padded edges), balanced by in-degree
   (LPT). Windows are processed in PAIRS: one gather instruction per table
   half per pair (~2300 rows each) amortizes the ~1us SWDGE fixed cost;
   gathers rotate across the 4 SWDGE queues (4 Q7 core pairs + descriptor
   rings), which overlaps descriptor generation with DMA drain.
 - Everything on device is bf16 (tolerance 2e-2; bf16 adds ~0.5%):
   PE matmuls run 4x faster and gather rows are 256B.
 - Node table rows are PARTITION-MAJOR: row(node) = slot*NTT + window, so
   phase T writes the table with one contiguous 2KB descriptor per
   partition per chunk instead of 256B/row descriptors, and a core's own
   rows [slot, ds(core*NWL+w)] load straight into SBUF with one DMA.
   Low/high table split (int16 gather indices) is slot<64 vs slot>=64.
 - Per pair, the one-hot S [edge, node] is built in ONE DVE op (iota vs
   dstloc broadcast); the transposed one-hot St [node, edge] (lhsT of the
   a_dst expansion matmul) is SHIPPED from host — it is pure index data,
   and DVE compare/broadcast ops run at ~1/4 copy speed, so a DMA is
   cheaper than rebuilding or PE-transposing (which needs a PSUM round
   trip of the same size).
 - Self-loops (PyG GATConv: loop edge_attr = per-dst mean of incoming
   edge_attr) fold in at window close from the unweighted aedge segment
   sum that rides the aggregation matmul.
"""

import math

import numpy as np

NCORES = 8
D_IN = 128
H_HEADS = 4
C_OUT = 16
HC = H_HEADS * C_OUT  # 64
ED_DIM = 64
NEG_SLOPE = 0.2
TW = 128             # table row width (bf16) -> 256B rows for dma_gather
UH = H_HEADS
MW = HC + 8          # M columns per block: [expal*xh | expal | a_edge0]

P = 128  # partitions / window slot count (127 real nodes + trash slot)

TRACE = False       # set by test harness to capture an NTFF profile
LAST_RESULT = None  # BassKernelResults of the last traced run


class _Cfg:
    def __init__(self, nwl, kl, kh, ncores):
        self.NWL = nwl            # windows per core (even)
        self.NPAIR = nwl // 2
        self.KL = kl              # low-half edge blocks per window
        self.KH = kh              # high-half edge blocks per window
        self.K = kl + kh          # 128-edge blocks per window
        self.K2 = 2 * self.K      # blocks per window pair
        self.EPP = self.K2 * P    # edge slots per window pair
        self.NTT = ncores * nwl   # global window count
        self.TPH = self.NTT // 2  # windows per table half (cores 0..3 = low)
        self.NT_PAD = self.NTT * P
        self.NSLOTS = self.NT_PAD
        self.SPLIT = P * self.TPH  # low-table rows
        self.ECB = nwl * self.K   # edge blocks per core

    def key(self):
        return (self.NWL, self.KL, self.KH, self.NTT)


def _fold_weights(W, W_edge, att_src, att_dst, att_edge):
    H, C = att_src.shape
    D = W.shape[1]
    ED = W_edge.shape[1]
    u_src = np.einsum("hc,hcd->hd", att_src, W.reshape(H, C, D))
    u_dst = np.einsum("hc,hcd->hd", att_dst, W.reshape(H, C, D))
    v = np.einsum("hc,hcd->hd", att_edge, W_edge.reshape(H, C, ED))
    # WallT columns = [W.T | u_src.T | u_dst.T | zero pad to TW]
    WallT = np.zeros((D, TW), np.float32)
    WallT[:, :HC] = W.T
    WallT[:, HC:HC + H] = u_src.T
    WallT[:, HC + H:HC + 2 * H] = u_dst.T
    # vT8: rows 0:ED -> [v.T | 0], rows ED:2ED -> [0 | v.T]  (paired matmul)
    vT8 = np.zeros((2 * ED, 2 * H), np.float32)
    vT8[:ED, :H] = v.T
    vT8[ED:, H:] = v.T
    return WallT, vT8


def _partition_nodes(dst, n_nodes, n_windows):
    """LPT-pack nodes into n_windows bins of <=127 nodes each (slot 127 is
    the trash slot), balancing in-degree sums."""
    import heapq

    cap = P - 1
    deg = np.bincount(dst, minlength=n_nodes).astype(np.int64)
    order = np.argsort(-deg, kind="stable")
    heap = [(0, w) for w in range(n_windows)]
    heapq.heapify(heap)
    win_of = np.empty(n_nodes, np.int32)
    slot_of = np.empty(n_nodes, np.int32)
    nodes_in = np.zeros(n_windows, np.int32)
    edges_in = np.zeros(n_windows, np.int64)
    for n in order:
        while True:
            e, w = heapq.heappop(heap)
            if nodes_in[w] < cap:
                break  # full windows are dropped from the heap for good
        win_of[n] = w
        slot_of[n] = nodes_in[w]
        nodes_in[w] += 1
        edges_in[w] += deg[n]
        if nodes_in[w] < cap:
            heapq.heappush(heap, (int(edges_in[w]), w))
    return win_of, slot_of


def _wrap16(idx, num):
    """int16 index array -> dma_gather layout: item i lives at partition
    i%16, col i//16; replicated down the remaining 112 partitions."""
    a = idx.astype(np.int16).reshape(num // 16, 16).T  # [16, num//16]
    return np.ascontiguousarray(np.tile(a, (8, 1)))


def _prep(x, src, dst, edge_attr, WallT, vT8):
    """Build per-core input maps + meta for unsharding."""
    import ml_dtypes
    bf = ml_dtypes.bfloat16

    n = x.shape[0]
    nwl = math.ceil(n / ((P - 1) * NCORES))
    if nwl % 2:
        nwl += 1  # windows are processed in pairs
    n_windows = NCORES * nwl

    win_of, slot_of = _partition_nodes(dst, n, n_windows)
    R_TRASH = P - 1

    winpos = win_of.astype(np.int64) * P + slot_of
    ntt = n_windows
    tph = ntt // 2
    # table row: half = (window >= tph); row = half*SPLIT + slot*tph + t_loc
    halfv = (win_of >= tph).astype(np.int64)
    row_of = (halfv * (P * tph) + slot_of.astype(np.int64) * tph
              + win_of - halfv * tph)
    split = P * tph
    assert split <= 32768 and ntt * P - split <= 32767

    ewin = win_of[dst]
    srow = row_of[src]
    is_low = (win_of[src] < tph)

    # fixed per-window low/high block counts across all cores (SPMD)
    nlow = np.bincount(ewin[is_low], minlength=n_windows)
    nhigh = np.bincount(ewin[~is_low], minlength=n_windows)
    kl = max(1, math.ceil(nlow.max() / P))
    kh = max(1, math.ceil(nhigh.max() / P))
    cfg = _Cfg(nwl, kl, kh, NCORES)
    K2, EPP = cfg.K2, cfg.EPP
    npair_g = n_windows // 2

    # ---- place edges pair-major: [low(2v) | low(2v+1) | hi(2v) | hi(2v+1)],
    #      each region padded to a block multiple ----
    pairg = ewin.astype(np.int64) // 2
    parity = ewin.astype(np.int64) % 2
    half = (~is_low).astype(np.int64)
    grp = pairg * 4 + half * 2 + parity
    order_e = np.argsort(grp, kind="stable")
    grp_s = grp[order_e]
    counts = np.bincount(grp_s, minlength=4 * npair_g)
    offs = np.zeros(4 * npair_g + 1, np.int64)
    np.cumsum(counts, out=offs[1:])
    pos = np.arange(len(order_e), dtype=np.int64) - offs[grp_s]
    roff = np.array([0, kl * P, 2 * kl * P, (2 * kl + kh) * P], np.int64)
    q = (grp_s // 4) * EPP + roff[grp_s % 4] + pos

    Q = npair_g * EPP
    lowmask_q = (np.arange(Q) % EPP) < 2 * kl * P
    gsrc_q = np.zeros(Q, np.int64)  # pads gather row 0 (harmless: trash dst)
    dstloc_q = np.full(Q, R_TRASH, np.int16)
    gsrc_q[q] = srow[order_e]
    dstloc_q[q] = slot_of[dst[order_e]].astype(np.int16)

    ea_q = np.zeros((Q, ED_DIM), np.float32)
    ea_q[q] = edge_attr[order_e]

    # node features in winpos (window-major) order: phase-T block t is
    # global window t, partition = slot
    x_ws = np.zeros((cfg.NT_PAD, D_IN), np.float32)
    x_ws[winpos] = x
    xT = np.ascontiguousarray(x_ws.T.astype(bf))  # [D_IN, NT_PAD]

    invcnt_ws = np.ones(n_windows * P, np.float32)
    cnt = np.bincount(dst, minlength=n).astype(np.float32)
    invcnt_ws[winpos] = 1.0 / np.maximum(cnt, 1.0)

    glow_q = np.where(lowmask_q, gsrc_q, 0)
    ghigh_q = np.where(lowmask_q, 0, np.maximum(gsrc_q - split, 0))
    assert glow_q.max() < split and ghigh_q.max() < ntt * P - split

    in_maps = []
    npair = cfg.NPAIR
    WallT16 = WallT.astype(bf)
    vT816 = vT8.astype(bf)
    slot_ar = np.arange(P, dtype=np.int16)
    for c in range(NCORES):
        qs, qe = c * npair * EPP, (c + 1) * npair * EPP
        dq = dstloc_q[qs:qe]
        eac = ea_q[qs:qe].reshape(npair * K2 // 2, 2, P, ED_DIM)
        eaT2 = np.ascontiguousarray(
            eac.transpose(1, 3, 0, 2).reshape(2 * ED_DIM, -1)).astype(bf)
        dstloc_c = np.ascontiguousarray(
            dq.reshape(npair * K2, P).T.astype(bf))   # [P, NPAIR*K2]
        StA = np.ascontiguousarray(
            (dq[None, :] == slot_ar[:, None]).astype(bf))  # [P, NPAIR*EPP]
        lo = glow_q[qs:qe].reshape(npair, EPP)
        hi = ghigh_q[qs:qe].reshape(npair, EPP)
        glo16 = np.concatenate(
            [_wrap16(lo[v, :2 * kl * P], 2 * kl * P) for v in range(npair)],
            axis=1)
        ghi16 = np.concatenate(
            [_wrap16(hi[v, 2 * kl * P:], 2 * kh * P) for v in range(npair)],
            axis=1)
        invcnt_c = np.ascontiguousarray(
            invcnt_ws[c * nwl * P:(c + 1) * nwl * P].reshape(nwl, P).T
            .astype(np.float32))
        selfbase = np.array([[(c // (NCORES // 2)) * split
                              + (c % (NCORES // 2)) * nwl]], np.uint32)
        in_maps.append(dict(
            xT=xT, eaT2=eaT2, dstloc=dstloc_c, StA=StA,
            invcnt=invcnt_c, glo16=glo16, ghi16=ghi16,
            WallT=WallT16, vT8=vT816, selfbase=selfbase,
        ))
    meta = dict(winpos=winpos, cfg=cfg)
    return cfg, in_maps, meta


def _build_nc(cfg):
    import concourse.bass as bass
    import concourse.tile as tile
    from concourse import bacc, mybir
    from contextlib import ExitStack

    f32 = mybir.dt.float32
    bf16 = mybir.dt.bfloat16
    i16 = mybir.dt.int16
    NWL, NPAIR, KL, KH = cfg.NWL, cfg.NPAIR, cfg.KL, cfg.KH
    K, K2, EPP = cfg.K, cfg.K2, cfg.EPP
    NTT, NT_PAD, SPLIT = cfg.NTT, cfg.NT_PAD, cfg.SPLIT
    TPH = cfg.TPH

    nc = bacc.Bacc("TRN2", target_bir_lowering=False, debug=False,
                   num_devices=NCORES, num_swdge_queues=4,
                   dynamic_dma_scratch_size=131072)
    xT = nc.dram_tensor("xT", [D_IN, NT_PAD], bf16, kind="ExternalInput").ap()
    WallT = nc.dram_tensor("WallT", [D_IN, TW], bf16,
                           kind="ExternalInput").ap()
    vT8 = nc.dram_tensor("vT8", [2 * ED_DIM, 2 * H_HEADS], bf16,
                         kind="ExternalInput").ap()
    eaT2 = nc.dram_tensor("eaT2", [2 * ED_DIM, NPAIR * K2 * P // 2], bf16,
                          kind="ExternalInput").ap()
    dstloc = nc.dram_tensor("dstloc", [P, NPAIR * K2], bf16,
                            kind="ExternalInput").ap()
    StA = nc.dram_tensor("StA", [P, NPAIR * EPP], bf16,
                         kind="ExternalInput").ap()
    invcnt = nc.dram_tensor("invcnt", [P, NWL], f32, kind="ExternalInput").ap()
    glo16 = nc.dram_tensor("glo16", [P, NPAIR * 2 * KL * 8], i16,
                           kind="ExternalInput").ap()
    ghi16 = nc.dram_tensor("ghi16", [P, NPAIR * 2 * KH * 8], i16,
                           kind="ExternalInput").ap()
    selfbase = nc.dram_tensor("selfbase", [1, 1], mybir.dt.uint32,
                              kind="ExternalInput").ap()
    out = nc.dram_tensor("out", [NWL * P, HC], f32, kind="ExternalOutput").ap()
    tableA = nc.dram_tensor("tableA", [NT_PAD + 4 * NWL, TW], bf16).ap()
    tabL3 = tableA[0:SPLIT, :].rearrange("(s t) u -> s t u", t=TPH)
    tabH3 = tableA[SPLIT:NT_PAD, :].rearrange("(s t) u -> s t u", t=TPH)

    with tile.TileContext(nc) as tc, ExitStack() as ctx:
        cpool = ctx.enter_context(tc.tile_pool(name="const", bufs=1))
        xpool = ctx.enter_context(tc.tile_pool(name="xload", bufs=2))
        tabpool = ctx.enter_context(tc.tile_pool(name="tab", bufs=2))
        eapool = ctx.enter_context(tc.tile_pool(name="ea", bufs=2))
        gpool = ctx.enter_context(tc.tile_pool(name="gather", bufs=2))
        ipool = ctx.enter_context(tc.tile_pool(name="idx", bufs=3))
        stpool = ctx.enter_context(tc.tile_pool(name="sT", bufs=2))
        spool = ctx.enter_context(tc.tile_pool(name="oneh", bufs=2))
        mpool = ctx.enter_context(tc.tile_pool(name="msg", bufs=2))
        wpool = ctx.enter_context(tc.tile_pool(name="work", bufs=2))
        opool = ctx.enter_context(tc.tile_pool(name="outw", bufs=1))
        pst = ctx.enter_context(tc.tile_pool(name="ps_t", bufs=2, space="PSUM"))
        pse = ctx.enter_context(tc.tile_pool(name="ps_e", bufs=2, space="PSUM"))
        psa = ctx.enter_context(tc.tile_pool(name="ps_a", bufs=2, space="PSUM"))
        psad = ctx.enter_context(tc.tile_pool(name="ps_ad", bufs=2,
                                              space="PSUM"))

        # ---- constants ----
        WallT_sb = cpool.tile([P, TW], bf16)
        nc.sync.dma_start(WallT_sb[:], WallT[:])
        vT8_sb = cpool.tile([2 * ED_DIM, 2 * H_HEADS], bf16)
        nc.sync.dma_start(vT8_sb[:], vT8[:])
        iota1 = cpool.tile([P, P], bf16)  # value = col index
        nc.gpsimd.iota(iota1[:], pattern=[[1, P]], base=0,
                       channel_multiplier=0,
                       allow_small_or_imprecise_dtypes=True)
        dstloc_sb = cpool.tile([P, NPAIR * K2], bf16)
        nc.sync.dma_start(dstloc_sb[:], dstloc[:])
        invcnt_sb = cpool.tile([P, NWL], f32)
        nc.sync.dma_start(invcnt_sb[:], invcnt[:])

        # ---- phase T: node table = [xh | a_src | a_dst | 0 pad] ----
        # low half (windows < TPH) first so low-table gathers start early
        XB = 8
        assert TPH % XB == 0
        for g in range(NTT // XB):
            t0 = g * XB
            tab3h = tabL3 if t0 < TPH else tabH3
            th0 = t0 if t0 < TPH else t0 - TPH
            xt = xpool.tile([P, XB * P], bf16, tag="xt")
            nc.sync.dma_start(xt[:], xT[:, t0 * P:(t0 + XB) * P])
            tab = tabpool.tile([P, XB * TW], bf16, tag="tab")
            for t4 in range(0, XB, 4):
                ps = pst.tile([P, 4 * TW], f32)
                for t in range(t4, t4 + 4):
                    nc.tensor.matmul(
                        out=ps[:, (t - t4) * TW:(t - t4 + 1) * TW],
                        lhsT=xt[:, t * P:(t + 1) * P],
                        rhs=WallT_sb[:], start=True, stop=True)
                nc.vector.tensor_copy(
                    tab[:, t4 * TW:(t4 + 4) * TW], ps[:])
            nc.scalar.dma_start(
                out=tab3h[:, th0:th0 + XB, :],
                in_=tab[:].rearrange("p (t u) -> p t u", u=TW))

        # ---- own node rows, straight into SBUF (per-core row base) ----
        selfall = cpool.tile([P, NWL * (HC + 8)], bf16)
        sreg = nc.sync.alloc_register("selfstart")
        nc.sync.reg_load(sreg, selfbase[0:1, 0:1])
        sstart = nc.sync.snap(sreg, donate=True, min_val=0,
                              max_val=SPLIT + (NCORES // 2 - 1) * NWL)
        nc.sync.dma_start(
            out=selfall[:].rearrange("p (w u) -> p w u", u=HC + 8),
            in_=tableA[bass.ds(sstart, P * TPH), 0:HC + 8]
            .rearrange("(s t) u -> s t u", t=TPH)[:, 0:NWL, :])

        # ---- phase B: per-pair attention softmax + aggregation ----
        # block j of a pair belongs to window parity blk_win[j]:
        blk_win = [0] * KL + [1] * KL + [0] * KH + [1] * KH
        win_blocks = [[j for j in range(K2) if blk_win[j] == e]
                      for e in (0, 1)]
        for v in range(NPAIR):
            gi = ipool.tile([P, 2 * (KL + KH) * 8], i16, tag="gi")
            nc.scalar.dma_start(gi[:, :2 * KL * 8],
                                glo16[:, v * 2 * KL * 8:(v + 1) * 2 * KL * 8])
            nc.scalar.dma_start(gi[:, 2 * KL * 8:],
                                ghi16[:, v * 2 * KH * 8:(v + 1) * 2 * KH * 8])
            G = gpool.tile([P, K2 * TW], bf16, tag="G")
            Gv = G[:].rearrange("p (k u) -> p k u", u=TW)
            nc.gpsimd.dma_gather(
                out_ap=Gv[:, 0:2 * KL, :], in_ap=tableA[0:SPLIT, :],
                idxs_ap=gi[:, 0:2 * KL * 8],
                num_idxs=2 * KL * P, num_idxs_reg=2 * KL * P, elem_size=TW,
                single_packet=False, queue_num=(2 * v) % 4)
            nc.gpsimd.dma_gather(
                out_ap=Gv[:, 2 * KL:K2, :], in_ap=tableA[SPLIT:NT_PAD, :],
                idxs_ap=gi[:, 2 * KL * 8:2 * (KL + KH) * 8],
                num_idxs=2 * KH * P, num_idxs_reg=2 * KH * P, elem_size=TW,
                single_packet=False, queue_num=(2 * v + 1) % 4)

            # transposed one-hot (lhsT of a_dst expansion): shipped from host
            St = stpool.tile([P, EPP], bf16, tag="St")
            nc.sync.dma_start(St[:], StA[:, v * EPP:(v + 1) * EPP])
            # one-hot S [edge, node]: one DVE op for the whole pair
            S = spool.tile([P, EPP], bf16, tag="S")
            nc.vector.tensor_tensor(
                out=S[:].rearrange("p (k u) -> p k u", u=P),
                in0=iota1[:].unsqueeze(1).broadcast_to([P, K2, P]),
                in1=dstloc_sb[:, v * K2:(v + 1) * K2].unsqueeze(2)
                .broadcast_to([P, K2, P]),
                op=mybir.AluOpType.is_equal)

            # a_dst(dst) per edge: St-block matmuls against own a_dst rows
            ps_adst = psad.tile([P, K2 * UH], f32)
            for j in range(K2):
                w = 2 * v + blk_win[j]
                nc.tensor.matmul(
                    out=ps_adst[:, j * UH:(j + 1) * UH],
                    lhsT=St[:, j * P:(j + 1) * P],
                    rhs=selfall[:, w * (HC + 8) + HC + 4:
                                w * (HC + 8) + HC + 8],
                    start=True, stop=True)

            # a_edge0 = edge_attr @ v.T for this pair (2 blocks per matmul)
            ea_ch = eapool.tile([2 * ED_DIM, K * P], bf16, tag="ea")
            nc.sync.dma_start(ea_ch[:], eaT2[:, v * K * P:(v + 1) * K * P])
            ps_e = pse.tile([P, K2 * UH], f32)
            for jj in range(K):
                nc.tensor.matmul(
                    out=ps_e[:, jj * 8:(jj + 1) * 8],
                    lhsT=ea_ch[:, jj * P:(jj + 1) * P],
                    rhs=vT8_sb[:], start=True, stop=True)

            # alpha = a_src(src) + a_dst(dst) + a_edge
            aw = wpool.tile([P, K2 * UH], f32, tag="aw")
            aw3 = aw[:].rearrange("p (k u) -> p k u", u=UH)
            nc.vector.tensor_tensor(
                out=aw3, in0=Gv[:, :, HC:HC + UH],
                in1=ps_adst[:].rearrange("p (k u) -> p k u", u=UH),
                op=mybir.AluOpType.add)
            nc.vector.tensor_tensor(
                out=aw[:], in0=aw[:], in1=ps_e[:], op=mybir.AluOpType.add)
            # lrelu(x) = slope*x + relu((1-slope)*x), then exp
            lrl = wpool.tile([P, K2 * UH], f32, tag="lrl")
            nc.scalar.activation(lrl[:], aw[:],
                                 mybir.ActivationFunctionType.Relu,
                                 scale=1.0 - NEG_SLOPE)
            nc.vector.scalar_tensor_tensor(
                out=lrl[:], in0=aw[:], scalar=NEG_SLOPE, in1=lrl[:],
                op0=mybir.AluOpType.mult, op1=mybir.AluOpType.add)

            # M = [expal * xh | expal | a_edge0] per block
            M = mpool.tile([P, K2 * MW], bf16, tag="M")
            M3 = M[:].rearrange("p (k u) -> p k u", u=MW)
            nc.scalar.activation(M3[:, :, HC:HC + UH],
                                 lrl[:].rearrange("p (k u) -> p k u", u=UH),
                                 mybir.ActivationFunctionType.Exp)
            nc.vector.tensor_copy(
                M3[:, :, HC + 4:HC + 8],
                ps_e[:].rearrange("p (k u) -> p k u", u=UH))
            expal_b = (M3[:, :, HC:HC + UH].unsqueeze(3)
                       .broadcast_to([P, K2, UH, C_OUT]))
            nc.vector.tensor_tensor(
                out=M3[:, :, 0:HC].rearrange("p k (h c) -> p k h c", c=C_OUT),
                in0=Gv[:, :, 0:HC].rearrange("p k (h c) -> p k h c", c=C_OUT),
                in1=expal_b, op=mybir.AluOpType.mult)

            # segment sums: one matmul per block, accumulated in PSUM;
            # both windows share one PSUM tile (disjoint column ranges)
            ps_agg = psa.tile([P, 2 * MW], f32)
            for e in (0, 1):
                blocks = win_blocks[e]
                for i, j in enumerate(blocks):
                    nc.tensor.matmul(
                        out=ps_agg[:, e * MW:(e + 1) * MW],
                        lhsT=S[:, j * P:(j + 1) * P],
                        rhs=M[:, j * MW:(j + 1) * MW],
                        start=(i == 0), stop=(i == len(blocks) - 1))

            # ---- window close: self-loop term + normalization ----
            for e in (0, 1):
                w = 2 * v + e
                agg = ps_agg[:, e * MW:(e + 1) * MW]
                selfr = selfall[:, w * (HC + 8):(w + 1) * (HC + 8)]
                lae = wpool.tile([P, 4], f32, tag=f"lae{e}")
                nc.vector.tensor_scalar(
                    out=lae[:], in0=agg[:, HC + 4:HC + 8],
                    scalar1=invcnt_sb[:, w:w + 1],
                    scalar2=None, op0=mybir.AluOpType.mult)
                asf = wpool.tile([P, 4], f32, tag=f"asf{e}")
                nc.vector.tensor_tensor(
                    out=asf[:], in0=selfr[:, HC:HC + 4],
                    in1=selfr[:, HC + 4:HC + 8], op=mybir.AluOpType.add)
                nc.vector.tensor_tensor(out=asf[:], in0=asf[:], in1=lae[:],
                                        op=mybir.AluOpType.add)
                es = wpool.tile([P, 4], f32, tag=f"es{e}")
                nc.scalar.activation(es[:], asf[:],
                                     mybir.ActivationFunctionType.Relu,
                                     scale=1.0 - NEG_SLOPE)
                nc.vector.scalar_tensor_tensor(
                    out=es[:], in0=asf[:], scalar=NEG_SLOPE, in1=es[:],
                    op0=mybir.AluOpType.mult, op1=mybir.AluOpType.add)
                nc.scalar.activation(es[:], es[:],
                                     mybir.ActivationFunctionType.Exp)
                # den = exp(alpha_self) + 1e-30 + sum_edges exp(alpha)
                den = wpool.tile([P, 4], f32, tag=f"den{e}")
                nc.vector.scalar_tensor_tensor(
                    out=den[:], in0=es[:], scalar=1e-30,
                    in1=agg[:, HC:HC + 4],
                    op0=mybir.AluOpType.add, op1=mybir.AluOpType.add)
                rec = wpool.tile([P, 4], f32, tag=f"rec{e}")
                nc.vector.reciprocal(rec[:], den[:])
                ot = opool.tile([P, HC], f32, tag=f"ot{e}")
                es_b = es[:].unsqueeze(2).broadcast_to([P, 4, C_OUT])
                nc.vector.tensor_tensor(
                    out=ot[:].rearrange("p (h c) -> p h c", c=C_OUT),
                    in0=selfr[:, 0:HC].rearrange("p (h c) -> p h c", c=C_OUT),
                    in1=es_b, op=mybir.AluOpType.mult)
                nc.vector.tensor_tensor(out=ot[:], in0=ot[:],
                                        in1=agg[:, 0:HC],
                                        op=mybir.AluOpType.add)
                rec_b = rec[:].unsqueeze(2).broadcast_to([P, 4, C_OUT])
                nc.vector.tensor_tensor(
                    out=ot[:].rearrange("p (h c) -> p h c", c=C_OUT),
                    in0=ot[:].rearrange("p (h c) -> p h c", c=C_OUT),
                    in1=rec_b, op=mybir.AluOpType.mult)
                nc.sync.dma_start(out[w * P:(w + 1) * P, :], ot[:])

    nc.compile()
    return nc


_NC_CACHE = {}


def _get_nc(cfg):
    k = cfg.key()
    if k not in _NC_CACHE:
        _NC_CACHE[k] = _build_nc(cfg)
    return _NC_CACHE[k]


def kernel(**inputs):
    x = np.asarray(inputs["x"], dtype=np.float32)
    ei = np.asarray(inputs["edge_index"])
    ea = np.asarray(inputs["edge_attr"], dtype=np.float32)
    W = np.asarray(inputs["W"], dtype=np.float32)
    W_edge = np.asarray(inputs["W_edge"], dtype=np.float32)
    att_src = np.asarray(inputs["att_src"], dtype=np.float32)
    att_dst = np.asarray(inputs["att_dst"], dtype=np.float32)
    att_edge = np.asarray(inputs["att_edge"], dtype=np.float32)
    bias = np.asarray(inputs["bias"], dtype=np.float32)

    src = ei[0].astype(np.int64)
    dst = ei[1].astype(np.int64)
    WallT, vT8 = _fold_weights(W, W_edge, att_src, att_dst, att_edge)

    cfg, in_maps, meta = _prep(x, src, dst, ea, WallT, vT8)
    nc = _get_nc(cfg)

    from concourse.bass_utils import run_bass_kernel_spmd
    res = run_bass_kernel_spmd(nc, in_maps, core_ids=list(range(NCORES)),
                               trace=TRACE)
    if TRACE:
        global LAST_RESULT
        LAST_RESULT = res

    out_ws = np.concatenate([res.results[c]["out"] for c in range(NCORES)],
                            axis=0)  # window-space [n_windows*P, HC]
    out = out_ws[meta["winpos"]]
    return (out + bias[None, :]).astype(np.float32)


# revision 17
# speedup vs baseline: 1.0192x; 1.0192x over previous
"""GAT message-passing kernel for Trainium2, 8 NeuronCores, dst-partitioned.

v3 (bf16, paired windows, shipped transposed one-hot, p-major table):
 - Fold attention vectors into the linear weights on host (tiny matmuls):
   a_src = x @ u_src.T, a_dst = x @ u_dst.T, a_edge = edge_attr @ v.T.
 - Softmax over incoming edges is computed WITHOUT max-subtraction (logits
   are bounded so exp cannot overflow; softmax is shift-invariant) so only
   segment-SUMS are needed, which map onto TensorE one-hot matmuls.
 - Host packs destination nodes into 128-slot windows (slot 127 of every
   window is a trash slot that absorbs padded edges), balanced by in-degree
   (LPT). Windows are processed in PAIRS: one gather instruction per table
   half per pair (~2300 rows each) amortizes the ~1us SWDGE fixed cost;
   gathers rotate across the 4 SWDGE queues (4 Q7 core pairs + descriptor
   rings), which overlaps descriptor generation with DMA drain.
 - Everything on device is bf16 (tolerance 2e-2; bf16 adds ~0.5%):
   PE matmuls run 4x faster and gather rows are 256B.
 - Node table rows are PARTITION-MAJOR: row(node) = slot*NTT + window, so
   phase T writes the table with one contiguous 2KB descriptor per
   partition per chunk instead of 256B/row descriptors, and a core's own
   rows [slot, ds(core*NWL+w)] load straight into SBUF with one DMA.
   Low/high table split (int16 gather indices) is slot<64 vs slot>=64.
 - Per pair, the one-hot S [edge, node] is built in ONE DVE op (iota vs
   dstloc broadcast); the transposed one-hot St [node, edge] (lhsT of the
   a_dst expansion matmul) is SHIPPED from host — it is pure index data,
   and DVE compare/broadcast ops run at ~1/4 copy speed, so a DMA is
   cheaper than rebuilding or PE-transposing (which needs a PSUM round
   trip of the same size).
 - Self-loops (PyG GATConv: loop edge_attr = per-dst mean of incoming
   edge_attr) fold in at window close from the unweighted aedge segment
   sum that rides the aggregation matmul.
"""

import math

import numpy as np

NCORES = 8
D_IN = 128
H_HEADS = 4
C_OUT = 16
HC = H_HEADS * C_OUT  # 64
ED_DIM = 64
NEG_SLOPE = 0.2
TW = 128             # table row width (bf16) -> 256B rows for dma_gather
UH = H_HEADS
MW = HC + 8          # M columns per block: [expal*xh | expal | a_edge0]

P = 128  # partitions / window slot count (127 real nodes + trash slot)

TRACE = False       # set by test harness to capture an NTFF profile
LAST_RESULT = None  # BassKernelResults of the last traced run


class _Cfg:
    def __init__(self, nwl, kl, kh, ncores):
        self.NWL = nwl            # windows per core (even)
        self.NPAIR = nwl // 2
        self.KL = kl              # low-half edge blocks per window
        self.KH = kh              # high-half edge blocks per window
        self.K = kl + kh          # 128-edge blocks per window
        self.K2 = 2 * self.K      # blocks per window pair
        self.EPP = self.K2 * P    # edge slots per window pair
        self.NTT = ncores * nwl   # global window count
        self.TPH = self.NTT // 2  # windows per table half (cores 0..3 = low)
        self.NT_PAD = self.NTT * P
        self.NSLOTS = self.NT_PAD
        self.SPLIT = P * self.TPH  # low-table rows
        self.ECB = nwl * self.K   # edge blocks per core

    def key(self):
        return (self.NWL, self.KL, self.KH, self.NTT)


def _fold_weights(W, W_edge, att_src, att_dst, att_edge):
    H, C = att_src.shape
    D = W.shape[1]
    ED = W_edge.shape[1]
    u_src = np.einsum("hc,hcd->hd", att_src, W.reshape(H, C, D))
    u_dst = np.einsum("hc,hcd->hd", att_dst, W.reshape(H, C, D))
    v = np.einsum("hc,hcd->hd", att_edge, W_edge.reshape(H, C, ED))
    # WallT columns = [W.T | u_src.T | u_dst.T | zero pad to TW]
    WallT = np.zeros((D, TW), np.float32)
    WallT[:, :HC] = W.T
    WallT[:, HC:HC + H] = u_src.T
    WallT[:, HC + H:HC + 2 * H] = u_dst.T
    # vT8: rows 0:ED -> [v.T | 0], rows ED:2ED -> [0 | v.T]  (paired matmul)
    vT8 = np.zeros((2 * ED, 2 * H), np.float32)
    vT8[:ED, :H] = v.T
    vT8[ED:, H:] = v.T
    return WallT, vT8


def _partition_nodes(dst, n_nodes, n_windows):
    """LPT-pack nodes into n_windows bins of <=127 nodes each (slot 127 is
    the trash slot), balancing in-degree sums."""
    import heapq

    cap = P - 1
    deg = np.bincount(dst, minlength=n_nodes).astype(np.int64)
    order = np.argsort(-deg, kind="stable")
    heap = [(0, w) for w in range(n_windows)]
    heapq.heapify(heap)
    win_of = np.empty(n_nodes, np.int32)
    slot_of = np.empty(n_nodes, np.int32)
    nodes_in = np.zeros(n_windows, np.int32)
    edges_in = np.zeros(n_windows, np.int64)
    for n in order:
        while True:
            e, w = heapq.heappop(heap)
            if nodes_in[w] < cap:
                break  # full windows are dropped from the heap for good
        win_of[n] = w
        slot_of[n] = nodes_in[w]
        nodes_in[w] += 1
        edges_in[w] += deg[n]
        if nodes_in[w] < cap:
            heapq.heappush(heap, (int(edges_in[w]), w))
    return win_of, slot_of


def _wrap16(idx, num):
    """int16 index array -> dma_gather layout: item i lives at partition
    i%16, col i//16; replicated down the remaining 112 partitions."""
    a = idx.astype(np.int16).reshape(num // 16, 16).T  # [16, num//16]
    return np.ascontiguousarray(np.tile(a, (8, 1)))


def _prep(x, src, dst, edge_attr, WallT, vT8):
    """Build per-core input maps + meta for unsharding."""
    import ml_dtypes
    bf = ml_dtypes.bfloat16

    n = x.shape[0]
    nwl = math.ceil(n / ((P - 1) * NCORES))
    if nwl % 2:
        nwl += 1  # windows are processed in pairs
    n_windows = NCORES * nwl

    win_of, slot_of = _partition_nodes(dst, n, n_windows)
    R_TRASH = P - 1

    winpos = win_of.astype(np.int64) * P + slot_of
    ntt = n_windows
    tph = ntt // 2
    # table row: half = (window >= tph); row = half*SPLIT + slot*tph + t_loc
    halfv = (win_of >= tph).astype(np.int64)
    row_of = (halfv * (P * tph) + slot_of.astype(np.int64) * tph
              + win_of - halfv * tph)
    split = P * tph
    assert split <= 32768 and ntt * P - split <= 32767

    ewin = win_of[dst]
    srow = row_of[src]
    is_low = (win_of[src] < tph)

    # fixed per-window low/high block counts across all cores (SPMD)
    nlow = np.bincount(ewin[is_low], minlength=n_windows)
    nhigh = np.bincount(ewin[~is_low], minlength=n_windows)
    kl = max(1, math.ceil(nlow.max() / P))
    kh = max(1, math.ceil(nhigh.max() / P))
    cfg = _Cfg(nwl, kl, kh, NCORES)
    K2, EPP = cfg.K2, cfg.EPP
    npair_g = n_windows // 2

    # ---- place edges pair-major: [low(2v) | low(2v+1) | hi(2v) | hi(2v+1)],
    #      each region padded to a block multiple ----
    pairg = ewin.astype(np.int64) // 2
    parity = ewin.astype(np.int64) % 2
    half = (~is_low).astype(np.int64)
    grp = pairg * 4 + half * 2 + parity
    order_e = np.argsort(grp, kind="stable")
    grp_s = grp[order_e]
    counts = np.bincount(grp_s, minlength=4 * npair_g)
    offs = np.zeros(4 * npair_g + 1, np.int64)
    np.cumsum(counts, out=offs[1:])
    pos = np.arange(len(order_e), dtype=np.int64) - offs[grp_s]
    roff = np.array([0, kl * P, 2 * kl * P, (2 * kl + kh) * P], np.int64)
    q = (grp_s // 4) * EPP + roff[grp_s % 4] + pos

    Q = npair_g * EPP
    lowmask_q = (np.arange(Q) % EPP) < 2 * kl * P
    gsrc_q = np.zeros(Q, np.int64)  # pads gather row 0 (harmless: trash dst)
    dstloc_q = np.full(Q, R_TRASH, np.int16)
    gsrc_q[q] = srow[order_e]
    dstloc_q[q] = slot_of[dst[order_e]].astype(np.int16)

    ea_q = np.zeros((Q, ED_DIM), np.float32)
    ea_q[q] = edge_attr[order_e]

    # node features in winpos (window-major) order: phase-T block t is
    # global window t, partition = slot
    x_ws = np.zeros((cfg.NT_PAD, D_IN), np.float32)
    x_ws[winpos] = x
    xT = np.ascontiguousarray(x_ws.T.astype(bf))  # [D_IN, NT_PAD]

    invcnt_ws = np.ones(n_windows * P, np.float32)
    cnt = np.bincount(dst, minlength=n).astype(np.float32)
    invcnt_ws[winpos] = 1.0 / np.maximum(cnt, 1.0)

    glow_q = np.where(lowmask_q, gsrc_q, 0)
    ghigh_q = np.where(lowmask_q, 0, np.maximum(gsrc_q - split, 0))
    assert glow_q.max() < split and ghigh_q.max() < ntt * P - split

    # window-major block order within a pair: [w0 low, w0 high, w1 low, w1 hi]
    kl2, kh2, K2c = kl, kh, K2
    blk_order = (list(range(0, kl2)) + list(range(2 * kl2, 2 * kl2 + kh2))
                 + list(range(kl2, 2 * kl2))
                 + list(range(2 * kl2 + kh2, K2c)))
    in_maps = []
    npair = cfg.NPAIR
    WallT16 = WallT.astype(bf)
    vT816 = vT8.astype(bf)
    slot_ar = np.arange(P, dtype=np.int16)
    for c in range(NCORES):
        qs, qe = c * npair * EPP, (c + 1) * npair * EPP
        dq = dstloc_q[qs:qe]
        eac = ea_q[qs:qe].reshape(npair * K2 // 2, 2, P, ED_DIM)
        eaT2 = np.ascontiguousarray(
            eac.transpose(1, 3, 0, 2).reshape(2 * ED_DIM, -1)).astype(bf)
        dq_re = dq.reshape(npair, K2, P)[:, blk_order, :].reshape(-1)
        dstloc_c = np.ascontiguousarray(
            dq_re.reshape(npair * K2, P).T.astype(bf))   # [P, NPAIR*K2]
        StA = np.ascontiguousarray(
            (dq_re[None, :] == slot_ar[:, None]).astype(bf))
        lo = glow_q[qs:qe].reshape(npair, EPP)
        hi = ghigh_q[qs:qe].reshape(npair, EPP)
        glo16 = np.concatenate(
            [_wrap16(lo[v, :2 * kl * P], 2 * kl * P) for v in range(npair)],
            axis=1)
        ghi16 = np.concatenate(
            [_wrap16(hi[v, 2 * kl * P:], 2 * kh * P) for v in range(npair)],
            axis=1)
        invcnt_c = np.ascontiguousarray(
            invcnt_ws[c * nwl * P:(c + 1) * nwl * P].reshape(nwl, P).T
            .astype(np.float32))
        selfbase = np.array([[(c // (NCORES // 2)) * split
                              + (c % (NCORES // 2)) * nwl]], np.uint32)
        in_maps.append(dict(
            xT=xT, eaT2=eaT2, dstloc=dstloc_c, StA=StA,
            invcnt=invcnt_c, glo16=glo16, ghi16=ghi16,
            WallT=WallT16, vT8=vT816, selfbase=selfbase,
        ))
    meta = dict(winpos=winpos, cfg=cfg)
    return cfg, in_maps, meta


def _build_nc(cfg):
    import concourse.bass as bass
    import concourse.tile as tile
    from concourse import bacc, mybir
    from contextlib import ExitStack

    f32 = mybir.dt.float32
    bf16 = mybir.dt.bfloat16
    i16 = mybir.dt.int16
    NWL, NPAIR, KL, KH = cfg.NWL, cfg.NPAIR, cfg.KL, cfg.KH
    K, K2, EPP = cfg.K, cfg.K2, cfg.EPP
    NTT, NT_PAD, SPLIT = cfg.NTT, cfg.NT_PAD, cfg.SPLIT
    TPH = cfg.TPH

    nc = bacc.Bacc("TRN2", target_bir_lowering=False, debug=False,
                   num_devices=NCORES, num_swdge_queues=4,
                   dynamic_dma_scratch_size=131072)
    xT = nc.dram_tensor("xT", [D_IN, NT_PAD], bf16, kind="ExternalInput").ap()
    WallT = nc.dram_tensor("WallT", [D_IN, TW], bf16,
                           kind="ExternalInput").ap()
    vT8 = nc.dram_tensor("vT8", [2 * ED_DIM, 2 * H_HEADS], bf16,
                         kind="ExternalInput").ap()
    eaT2 = nc.dram_tensor("eaT2", [2 * ED_DIM, NPAIR * K2 * P // 2], bf16,
                          kind="ExternalInput").ap()
    dstloc = nc.dram_tensor("dstloc", [P, NPAIR * K2], bf16,
                            kind="ExternalInput").ap()
    StA = nc.dram_tensor("StA", [P, NPAIR * EPP], bf16,
                         kind="ExternalInput").ap()
    invcnt = nc.dram_tensor("invcnt", [P, NWL], f32, kind="ExternalInput").ap()
    glo16 = nc.dram_tensor("glo16", [P, NPAIR * 2 * KL * 8], i16,
                           kind="ExternalInput").ap()
    ghi16 = nc.dram_tensor("ghi16", [P, NPAIR * 2 * KH * 8], i16,
                           kind="ExternalInput").ap()
    selfbase = nc.dram_tensor("selfbase", [1, 1], mybir.dt.uint32,
                              kind="ExternalInput").ap()
    out = nc.dram_tensor("out", [NWL * P, HC], f32, kind="ExternalOutput").ap()
    tableA = nc.dram_tensor("tableA", [NT_PAD + 4 * NWL, TW], bf16).ap()
    tabL3 = tableA[0:SPLIT, :].rearrange("(s t) u -> s t u", t=TPH)
    tabH3 = tableA[SPLIT:NT_PAD, :].rearrange("(s t) u -> s t u", t=TPH)

    with tile.TileContext(nc) as tc, ExitStack() as ctx:
        cpool = ctx.enter_context(tc.tile_pool(name="const", bufs=1))
        xpool = ctx.enter_context(tc.tile_pool(name="xload", bufs=2))
        tabpool = ctx.enter_context(tc.tile_pool(name="tab", bufs=2))
        eapool = ctx.enter_context(tc.tile_pool(name="ea", bufs=2))
        gpool = ctx.enter_context(tc.tile_pool(name="gather", bufs=3))
        ipool = ctx.enter_context(tc.tile_pool(name="idx", bufs=3))
        stpool = ctx.enter_context(tc.tile_pool(name="sT", bufs=1))
        spool = ctx.enter_context(tc.tile_pool(name="oneh", bufs=1))
        mpool = ctx.enter_context(tc.tile_pool(name="msg", bufs=2))
        wpool = ctx.enter_context(tc.tile_pool(name="work", bufs=3))
        opool = ctx.enter_context(tc.tile_pool(name="outw", bufs=3))
        pst = ctx.enter_context(tc.tile_pool(name="ps_t", bufs=2, space="PSUM"))
        pse = ctx.enter_context(tc.tile_pool(name="ps_e", bufs=2, space="PSUM"))
        psa = ctx.enter_context(tc.tile_pool(name="ps_a", bufs=2, space="PSUM"))
        psad = ctx.enter_context(tc.tile_pool(name="ps_ad", bufs=2,
                                              space="PSUM"))

        # ---- constants ----
        WallT_sb = cpool.tile([P, TW], bf16)
        nc.sync.dma_start(WallT_sb[:], WallT[:])
        vT8_sb = cpool.tile([2 * ED_DIM, 2 * H_HEADS], bf16)
        nc.sync.dma_start(vT8_sb[:], vT8[:])
        iota1 = cpool.tile([P, P], bf16)  # value = col index
        nc.gpsimd.iota(iota1[:], pattern=[[1, P]], base=0,
                       channel_multiplier=0,
                       allow_small_or_imprecise_dtypes=True)
        dstloc_sb = cpool.tile([P, NPAIR * K2], bf16)
        nc.sync.dma_start(dstloc_sb[:], dstloc[:])
        invcnt_sb = cpool.tile([P, NWL], f32)
        nc.sync.dma_start(invcnt_sb[:], invcnt[:])

        # ---- phase T: node table = [xh | a_src | a_dst | 0 pad] ----
        # low half (windows < TPH) first so low-table gathers start early
        XB = 8
        assert TPH % XB == 0
        for g in range(NTT // XB):
            t0 = g * XB
            tab3h = tabL3 if t0 < TPH else tabH3
            th0 = t0 if t0 < TPH else t0 - TPH
            xt = xpool.tile([P, XB * P], bf16, tag="xt")
            nc.sync.dma_start(xt[:], xT[:, t0 * P:(t0 + XB) * P])
            tab = tabpool.tile([P, XB * TW], bf16, tag="tab")
            for t4 in range(0, XB, 4):
                ps = pst.tile([P, 4 * TW], f32)
                for t in range(t4, t4 + 4):
                    nc.tensor.matmul(
                        out=ps[:, (t - t4) * TW:(t - t4 + 1) * TW],
                        lhsT=xt[:, t * P:(t + 1) * P],
                        rhs=WallT_sb[:], start=True, stop=True)
                nc.vector.tensor_copy(
                    tab[:, t4 * TW:(t4 + 4) * TW], ps[:])
            nc.scalar.dma_start(
                out=tab3h[:, th0:th0 + XB, :],
                in_=tab[:].rearrange("p (t u) -> p t u", u=TW))

        # ---- own node rows, straight into SBUF (per-core row base) ----
        selfall = cpool.tile([P, NWL * (HC + 8)], bf16)
        sreg = nc.sync.alloc_register("selfstart")
        nc.sync.reg_load(sreg, selfbase[0:1, 0:1])
        sstart = nc.sync.snap(sreg, donate=True, min_val=0,
                              max_val=SPLIT + (NCORES // 2 - 1) * NWL)
        nc.sync.dma_start(
            out=selfall[:].rearrange("p (w u) -> p w u", u=HC + 8),
            in_=tableA[bass.ds(sstart, P * TPH), 0:HC + 8]
            .rearrange("(s t) u -> s t u", t=TPH)[:, 0:NWL, :])

        # ---- phase B: per-pair attention softmax + aggregation ----
        # block j of a pair belongs to window parity blk_win[j]:
        blk_win = [0] * KL + [1] * KL + [0] * KH + [1] * KH
        win_blocks = [[j for j in range(K2) if blk_win[j] == e]
                      for e in (0, 1)]
        for v in range(NPAIR):
            gi = ipool.tile([P, 2 * (KL + KH) * 8], i16, tag="gi")
            nc.scalar.dma_start(gi[:, :2 * KL * 8],
                                glo16[:, v * 2 * KL * 8:(v + 1) * 2 * KL * 8])
            nc.scalar.dma_start(gi[:, 2 * KL * 8:],
                                ghi16[:, v * 2 * KH * 8:(v + 1) * 2 * KH * 8])
            G = gpool.tile([P, K2 * TW], bf16, tag="G")
            Gv = G[:].rearrange("p (k u) -> p k u", u=TW)
            nc.gpsimd.dma_gather(
                out_ap=Gv[:, 0:2 * KL, :], in_ap=tableA[0:SPLIT, :],
                idxs_ap=gi[:, 0:2 * KL * 8],
                num_idxs=2 * KL * P, num_idxs_reg=2 * KL * P, elem_size=TW,
                single_packet=False, queue_num=(2 * v) % 4)
            nc.gpsimd.dma_gather(
                out_ap=Gv[:, 2 * KL:K2, :], in_ap=tableA[SPLIT:NT_PAD, :],
                idxs_ap=gi[:, 2 * KL * 8:2 * (KL + KH) * 8],
                num_idxs=2 * KH * P, num_idxs_reg=2 * KH * P, elem_size=TW,
                single_packet=False, queue_num=(2 * v + 1) % 4)

            # transposed one-hot (lhsT of a_dst expansion): shipped from
            # host in window-major block order; one-hot S built per window
            ps_adst = psad.tile([P, K2 * UH], f32)
            Sw = []
            for e in (0, 1):
                w = 2 * v + e
                St = stpool.tile([P, K * P], bf16, tag=f"St{e}")
                nc.sync.dma_start(
                    St[:], StA[:, (2 * v + e) * K * P:(2 * v + e + 1) * K * P])
                S = spool.tile([P, K * P], bf16, tag=f"S{e}")
                nc.vector.tensor_tensor(
                    out=S[:].rearrange("p (k u) -> p k u", u=P),
                    in0=iota1[:].unsqueeze(1).broadcast_to([P, K, P]),
                    in1=dstloc_sb[:, (2 * v + e) * K:(2 * v + e + 1) * K]
                    .unsqueeze(2).broadcast_to([P, K, P]),
                    op=mybir.AluOpType.is_equal)
                Sw.append(S)
                for i, j in enumerate(win_blocks[e]):
                    nc.tensor.matmul(
                        out=ps_adst[:, j * UH:(j + 1) * UH],
                        lhsT=St[:, i * P:(i + 1) * P],
                        rhs=selfall[:, w * (HC + 8) + HC + 4:
                                    w * (HC + 8) + HC + 8],
                        start=True, stop=True)

            # a_edge0 = edge_attr @ v.T for this pair (2 blocks per matmul)
            ea_ch = eapool.tile([2 * ED_DIM, K * P], bf16, tag="ea")
            nc.sync.dma_start(ea_ch[:], eaT2[:, v * K * P:(v + 1) * K * P])
            ps_e = pse.tile([P, K2 * UH], f32)
            for jj in range(K):
                nc.tensor.matmul(
                    out=ps_e[:, jj * 8:(jj + 1) * 8],
                    lhsT=ea_ch[:, jj * P:(jj + 1) * P],
                    rhs=vT8_sb[:], start=True, stop=True)

            # alpha = a_src(src) + a_dst(dst) + a_edge
            aw = wpool.tile([P, K2 * UH], f32, tag="aw")
            aw3 = aw[:].rearrange("p (k u) -> p k u", u=UH)
            nc.vector.tensor_tensor(
                out=aw3, in0=Gv[:, :, HC:HC + UH],
                in1=ps_adst[:].rearrange("p (k u) -> p k u", u=UH),
                op=mybir.AluOpType.add)
            nc.vector.tensor_tensor(
                out=aw[:], in0=aw[:], in1=ps_e[:], op=mybir.AluOpType.add)
            # lrelu(x) = slope*x + relu((1-slope)*x), then exp
            lrl = wpool.tile([P, K2 * UH], f32, tag="lrl")
            nc.scalar.activation(lrl[:], aw[:],
                                 mybir.ActivationFunctionType.Relu,
                                 scale=1.0 - NEG_SLOPE)
            nc.vector.scalar_tensor_tensor(
                out=lrl[:], in0=aw[:], scalar=NEG_SLOPE, in1=lrl[:],
                op0=mybir.AluOpType.mult, op1=mybir.AluOpType.add)

            # M = [expal * xh | expal | a_edge0] per block
            M = mpool.tile([P, K2 * MW], bf16, tag="M")
            M3 = M[:].rearrange("p (k u) -> p k u", u=MW)
            nc.scalar.activation(M3[:, :, HC:HC + UH],
                                 lrl[:].rearrange("p (k u) -> p k u", u=UH),
                                 mybir.ActivationFunctionType.Exp)
            nc.vector.tensor_copy(
                M3[:, :, HC + 4:HC + 8],
                ps_e[:].rearrange("p (k u) -> p k u", u=UH))
            expal_b = (M3[:, :, HC:HC + UH].unsqueeze(3)
                       .broadcast_to([P, K2, UH, C_OUT]))
            nc.vector.tensor_tensor(
                out=M3[:, :, 0:HC].rearrange("p k (h c) -> p k h c", c=C_OUT),
                in0=Gv[:, :, 0:HC].rearrange("p k (h c) -> p k h c", c=C_OUT),
                in1=expal_b, op=mybir.AluOpType.mult)

            # segment sums: one matmul per block, accumulated in PSUM;
            # both windows share one PSUM tile (disjoint column ranges)
            ps_agg = psa.tile([P, 2 * MW], f32)
            for e in (0, 1):
                blocks = win_blocks[e]
                for i, j in enumerate(blocks):
                    nc.tensor.matmul(
                        out=ps_agg[:, e * MW:(e + 1) * MW],
                        lhsT=Sw[e][:, i * P:(i + 1) * P],
                        rhs=M[:, j * MW:(j + 1) * MW],
                        start=(i == 0), stop=(i == len(blocks) - 1))

            # ---- window close: self-loop term + normalization ----
            for e in (0, 1):
                w = 2 * v + e
                agg = ps_agg[:, e * MW:(e + 1) * MW]
                selfr = selfall[:, w * (HC + 8):(w + 1) * (HC + 8)]
                lae = wpool.tile([P, 4], f32, tag=f"lae{e}")
                nc.vector.tensor_scalar(
                    out=lae[:], in0=agg[:, HC + 4:HC + 8],
                    scalar1=invcnt_sb[:, w:w + 1],
                    scalar2=None, op0=mybir.AluOpType.mult)
                asf = wpool.tile([P, 4], f32, tag=f"asf{e}")
                nc.vector.tensor_tensor(
                    out=asf[:], in0=selfr[:, HC:HC + 4],
                    in1=selfr[:, HC + 4:HC + 8], op=mybir.AluOpType.add)
                nc.vector.tensor_tensor(out=asf[:], in0=asf[:], in1=lae[:],
                                        op=mybir.AluOpType.add)
                es = wpool.tile([P, 4], f32, tag=f"es{e}")
                nc.scalar.activation(es[:], asf[:],
                                     mybir.ActivationFunctionType.Relu,
                                     scale=1.0 - NEG_SLOPE)
                nc.vector.scalar_tensor_tensor(
                    out=es[:], in0=asf[:], scalar=NEG_SLOPE, in1=es[:],
                    op0=mybir.AluOpType.mult, op1=mybir.AluOpType.add)
                nc.scalar.activation(es[:], es[:],
                                     mybir.ActivationFunctionType.Exp)
                # den = exp(alpha_self) + 1e-30 + sum_edges exp(alpha)
                den = wpool.tile([P, 4], f32, tag=f"den{e}")
                nc.vector.scalar_tensor_tensor(
                    out=den[:], in0=es[:], scalar=1e-30,
                    in1=agg[:, HC:HC + 4],
                    op0=mybir.AluOpType.add, op1=mybir.AluOpType.add)
                rec = wpool.tile([P, 4], f32, tag=f"rec{e}")
                nc.vector.reciprocal(rec[:], den[:])
                ot = opool.tile([P, HC], f32, tag=f"ot{e}")
                es_b = es[:].unsqueeze(2).broadcast_to([P, 4, C_OUT])
                nc.vector.tensor_tensor(
                    out=ot[:].rearrange("p (h c) -> p h c", c=C_OUT),
                    in0=selfr[:, 0:HC].rearrange("p (h c) -> p h c", c=C_OUT),
                    in1=es_b, op=mybir.AluOpType.mult)
                nc.vector.tensor_tensor(out=ot[:], in0=ot[:],
                                        in1=agg[:, 0:HC],
                                        op=mybir.AluOpType.add)
                rec_b = rec[:].unsqueeze(2).broadcast_to([P, 4, C_OUT])
                nc.vector.tensor_tensor(
                    out=ot[:].rearrange("p (h c) -> p h c", c=C_OUT),
                    in0=ot[:].rearrange("p (h c) -> p h c", c=C_OUT),
                    in1=rec_b, op=mybir.AluOpType.mult)
                nc.sync.dma_start(out[w * P:(w + 1) * P, :], ot[:])

    nc.compile()
    return nc


_NC_CACHE = {}


def _get_nc(cfg):
    k = cfg.key()
    if k not in _NC_CACHE:
        _NC_CACHE[k] = _build_nc(cfg)
    return _NC_CACHE[k]


def kernel(**inputs):
    x = np.asarray(inputs["x"], dtype=np.float32)
    ei = np.asarray(inputs["edge_index"])
    ea = np.asarray(inputs["edge_attr"], dtype=np.float32)
    W = np.asarray(inputs["W"], dtype=np.float32)
    W_edge = np.asarray(inputs["W_edge"], dtype=np.float32)
    att_src = np.asarray(inputs["att_src"], dtype=np.float32)
    att_dst = np.asarray(inputs["att_dst"], dtype=np.float32)
    att_edge = np.asarray(inputs["att_edge"], dtype=np.float32)
    bias = np.asarray(inputs["bias"], dtype=np.float32)

    src = ei[0].astype(np.int64)
    dst = ei[1].astype(np.int64)
    WallT, vT8 = _fold_weights(W, W_edge, att_src, att_dst, att_edge)

    cfg, in_maps, meta = _prep(x, src, dst, ea, WallT, vT8)
    nc = _get_nc(cfg)

    from concourse.bass_utils import run_bass_kernel_spmd
    res = run_bass_kernel_spmd(nc, in_maps, core_ids=list(range(NCORES)),
                               trace=TRACE)
    if TRACE:
        global LAST_RESULT
        LAST_RESULT = res

    out_ws = np.concatenate([res.results[c]["out"] for c in range(NCORES)],
                            axis=0)  # window-space [n_windows*P, HC]
    out = out_ws[meta["winpos"]]
    return (out + bias[None, :]).astype(np.float32)


# revision 18
# speedup vs baseline: 1.4239x; 1.3972x over previous
"""GAT message-passing kernel for Trainium2, 8 NeuronCores, dst-partitioned.

v3 (bf16, paired windows, shipped transposed one-hot, p-major table):
 - Fold attention vectors into the linear weights on host (tiny matmuls):
   a_src = x @ u_src.T, a_dst = x @ u_dst.T, a_edge = edge_attr @ v.T.
 - Softmax over incoming edges is computed WITHOUT max-subtraction (logits
   are bounded so exp cannot overflow; softmax is shift-invariant) so only
   segment-SUMS are needed, which map onto TensorE one-hot matmuls.
 - Host packs destination nodes into 128-slot windows (slot 127 of every
   window is a trash slot that absorbs padded edges), balanced by in-degree
   (LPT). Windows are processed in PAIRS: one gather instruction per table
   half per pair (~2300 rows each) amortizes the ~1us SWDGE fixed cost;
   gathers rotate across the 4 SWDGE queues (4 Q7 core pairs + descriptor
   rings), which overlaps descriptor generation with DMA drain.
 - Everything on device is bf16 (tolerance 2e-2; bf16 adds ~0.5%):
   PE matmuls run 4x faster and gather rows are 256B.
 - Node table rows are PARTITION-MAJOR: row(node) = slot*NTT + window, so
   phase T writes the table with one contiguous 2KB descriptor per
   partition per chunk instead of 256B/row descriptors, and a core's own
   rows [slot, ds(core*NWL+w)] load straight into SBUF with one DMA.
   Low/high table split (int16 gather indices) is slot<64 vs slot>=64.
 - Per pair, the one-hot S [edge, node] is built in ONE DVE op (iota vs
   dstloc broadcast); the transposed one-hot St [node, edge] (lhsT of the
   a_dst expansion matmul) is SHIPPED from host — it is pure index data,
   and DVE compare/broadcast ops run at ~1/4 copy speed, so a DMA is
   cheaper than rebuilding or PE-transposing (which needs a PSUM round
   trip of the same size).
 - Self-loops (PyG GATConv: loop edge_attr = per-dst mean of incoming
   edge_attr) fold in at window close from the unweighted aedge segment
   sum that rides the aggregation matmul.
"""

import math

import numpy as np

NCORES = 8
D_IN = 128
H_HEADS = 4
C_OUT = 16
HC = H_HEADS * C_OUT  # 64
ED_DIM = 64
NEG_SLOPE = 0.2
TW = 128             # table row width (bf16) -> 256B rows for dma_gather
UH = H_HEADS
MW = HC + 8          # M columns per block: [expal*xh | expal | a_edge0]

P = 128  # partitions / window slot count (127 real nodes + trash slot)

TRACE = False       # set by test harness to capture an NTFF profile
LAST_RESULT = None  # BassKernelResults of the last traced run


class _Cfg:
    def __init__(self, nwl, kl, kh, ncores):
        self.NWL = nwl            # windows per core (even)
        self.NPAIR = nwl // 2
        self.KL = kl              # low-half edge blocks per window
        self.KH = kh              # high-half edge blocks per window
        self.K = kl + kh          # 128-edge blocks per window
        self.K2 = 2 * self.K      # blocks per window pair
        self.EPP = self.K2 * P    # edge slots per window pair
        self.NTT = ncores * nwl   # global window count
        self.TPH = self.NTT // 2  # windows per table half (cores 0..3 = low)
        self.NT_PAD = self.NTT * P
        self.NSLOTS = self.NT_PAD
        self.SPLIT = P * self.TPH  # low-table rows
        self.ECB = nwl * self.K   # edge blocks per core

    def key(self):
        return (self.NWL, self.KL, self.KH, self.NTT)


def _fold_weights(W, W_edge, att_src, att_dst, att_edge):
    H, C = att_src.shape
    D = W.shape[1]
    ED = W_edge.shape[1]
    u_src = np.einsum("hc,hcd->hd", att_src, W.reshape(H, C, D))
    u_dst = np.einsum("hc,hcd->hd", att_dst, W.reshape(H, C, D))
    v = np.einsum("hc,hcd->hd", att_edge, W_edge.reshape(H, C, ED))
    # WallT columns = [W.T | u_src.T | u_dst.T | zero pad to TW]
    WallT = np.zeros((D, TW), np.float32)
    WallT[:, :HC] = W.T
    WallT[:, HC:HC + H] = u_src.T
    WallT[:, HC + H:HC + 2 * H] = u_dst.T
    # vT8: rows 0:ED -> [v.T | 0], rows ED:2ED -> [0 | v.T]  (paired matmul)
    vT8 = np.zeros((2 * ED, 2 * H), np.float32)
    vT8[:ED, :H] = v.T
    vT8[ED:, H:] = v.T
    return WallT, vT8


def _partition_nodes(dst, n_nodes, n_windows):
    """LPT-pack nodes into n_windows bins of <=127 nodes each (slot 127 is
    the trash slot), balancing in-degree sums."""
    import heapq

    cap = P - 1
    deg = np.bincount(dst, minlength=n_nodes).astype(np.int64)
    order = np.argsort(-deg, kind="stable")
    heap = [(0, w) for w in range(n_windows)]
    heapq.heapify(heap)
    win_of = np.empty(n_nodes, np.int32)
    slot_of = np.empty(n_nodes, np.int32)
    nodes_in = np.zeros(n_windows, np.int32)
    edges_in = np.zeros(n_windows, np.int64)
    for n in order:
        while True:
            e, w = heapq.heappop(heap)
            if nodes_in[w] < cap:
                break  # full windows are dropped from the heap for good
        win_of[n] = w
        slot_of[n] = nodes_in[w]
        nodes_in[w] += 1
        edges_in[w] += deg[n]
        if nodes_in[w] < cap:
            heapq.heappush(heap, (int(edges_in[w]), w))
    return win_of, slot_of


def _wrap16(idx, num):
    """int16 index array -> dma_gather layout: item i lives at partition
    i%16, col i//16; replicated down the remaining 112 partitions."""
    a = idx.astype(np.int16).reshape(num // 16, 16).T  # [16, num//16]
    return np.ascontiguousarray(np.tile(a, (8, 1)))


def _prep(x, src, dst, edge_attr, WallT, vT8):
    """Build per-core input maps + meta for unsharding."""
    import ml_dtypes
    bf = ml_dtypes.bfloat16

    n = x.shape[0]
    nwl = math.ceil(n / ((P - 1) * NCORES))
    if nwl % 2:
        nwl += 1  # windows are processed in pairs
    n_windows = NCORES * nwl

    win_of, slot_of = _partition_nodes(dst, n, n_windows)
    R_TRASH = P - 1

    winpos = win_of.astype(np.int64) * P + slot_of
    ntt = n_windows
    tph = ntt // 2
    # table row: half = (window >= tph); row = half*SPLIT + slot*tph + t_loc
    halfv = (win_of >= tph).astype(np.int64)
    row_of = (halfv * (P * tph) + slot_of.astype(np.int64) * tph
              + win_of - halfv * tph)
    split = P * tph
    assert split <= 32768 and ntt * P - split <= 32767

    ewin = win_of[dst]
    srow = row_of[src]
    is_low = (win_of[src] < tph)

    # fixed per-window low/high block counts across all cores (SPMD)
    nlow = np.bincount(ewin[is_low], minlength=n_windows)
    nhigh = np.bincount(ewin[~is_low], minlength=n_windows)
    kl = max(1, math.ceil(nlow.max() / P))
    kh = max(1, math.ceil(nhigh.max() / P))
    cfg = _Cfg(nwl, kl, kh, NCORES)
    K2, EPP = cfg.K2, cfg.EPP
    npair_g = n_windows // 2

    # ---- place edges pair-major: [low(2v) | low(2v+1) | hi(2v) | hi(2v+1)],
    #      each region padded to a block multiple ----
    pairg = ewin.astype(np.int64) // 2
    parity = ewin.astype(np.int64) % 2
    half = (~is_low).astype(np.int64)
    grp = pairg * 4 + half * 2 + parity
    order_e = np.argsort(grp, kind="stable")
    grp_s = grp[order_e]
    counts = np.bincount(grp_s, minlength=4 * npair_g)
    offs = np.zeros(4 * npair_g + 1, np.int64)
    np.cumsum(counts, out=offs[1:])
    pos = np.arange(len(order_e), dtype=np.int64) - offs[grp_s]
    roff = np.array([0, kl * P, 2 * kl * P, (2 * kl + kh) * P], np.int64)
    q = (grp_s // 4) * EPP + roff[grp_s % 4] + pos

    Q = npair_g * EPP
    lowmask_q = (np.arange(Q) % EPP) < 2 * kl * P
    gsrc_q = np.zeros(Q, np.int64)  # pads gather row 0 (harmless: trash dst)
    dstloc_q = np.full(Q, R_TRASH, np.int16)
    gsrc_q[q] = srow[order_e]
    dstloc_q[q] = slot_of[dst[order_e]].astype(np.int16)

    ea_q = np.zeros((Q, ED_DIM), np.float32)
    ea_q[q] = edge_attr[order_e]

    # node features in winpos (window-major) order: phase-T block t is
    # global window t, partition = slot
    x_ws = np.zeros((cfg.NT_PAD, D_IN), np.float32)
    x_ws[winpos] = x
    xT = np.ascontiguousarray(x_ws.T.astype(bf))  # [D_IN, NT_PAD]

    invcnt_ws = np.ones(n_windows * P, np.float32)
    cnt = np.bincount(dst, minlength=n).astype(np.float32)
    invcnt_ws[winpos] = 1.0 / np.maximum(cnt, 1.0)

    glow_q = np.where(lowmask_q, gsrc_q, 0)
    ghigh_q = np.where(lowmask_q, 0, np.maximum(gsrc_q - split, 0))
    assert glow_q.max() < split and ghigh_q.max() < ntt * P - split

    # window-major block order within a pair: [w0 low, w0 high, w1 low, w1 hi]
    kl2, kh2, K2c = kl, kh, K2
    blk_order = (list(range(0, kl2)) + list(range(2 * kl2, 2 * kl2 + kh2))
                 + list(range(kl2, 2 * kl2))
                 + list(range(2 * kl2 + kh2, K2c)))
    in_maps = []
    npair = cfg.NPAIR
    WallT16 = WallT.astype(bf)
    vT816 = vT8.astype(bf)
    slot_ar = np.arange(P, dtype=np.int16)
    for c in range(NCORES):
        qs, qe = c * npair * EPP, (c + 1) * npair * EPP
        dq = dstloc_q[qs:qe]
        eac = ea_q[qs:qe].reshape(npair * K2 // 2, 2, P, ED_DIM)
        eaT2 = np.ascontiguousarray(
            eac.transpose(1, 3, 0, 2).reshape(2 * ED_DIM, -1)).astype(bf)
        dq_re = dq.reshape(npair, K2, P)[:, blk_order, :].reshape(-1)
        dstloc_c = np.ascontiguousarray(
            dq_re.reshape(npair * K2, P).T.astype(bf))   # [P, NPAIR*K2]
        StA = np.ascontiguousarray(
            (dq_re[None, :] == slot_ar[:, None]).astype(bf))
        lo = glow_q[qs:qe].reshape(npair, EPP)
        hi = ghigh_q[qs:qe].reshape(npair, EPP)
        glo16 = np.concatenate(
            [_wrap16(lo[v, :2 * kl * P], 2 * kl * P) for v in range(npair)],
            axis=1)
        ghi16 = np.concatenate(
            [_wrap16(hi[v, 2 * kl * P:], 2 * kh * P) for v in range(npair)],
            axis=1)
        invcnt_c = np.ascontiguousarray(
            invcnt_ws[c * nwl * P:(c + 1) * nwl * P].reshape(nwl, P).T
            .astype(np.float32))
        selfbase = np.array([[(c // (NCORES // 2)) * split
                              + (c % (NCORES // 2)) * nwl]], np.uint32)
        in_maps.append(dict(
            xT=xT, eaT2=eaT2, dstloc=dstloc_c, StA=StA,
            invcnt=invcnt_c, glo16=glo16, ghi16=ghi16,
            WallT=WallT16, vT8=vT816, selfbase=selfbase,
        ))
    meta = dict(winpos=winpos, cfg=cfg)
    return cfg, in_maps, meta


def _build_nc(cfg):
    import concourse.bass as bass
    import concourse.tile as tile
    from concourse import bacc, mybir
    from contextlib import ExitStack

    f32 = mybir.dt.float32
    bf16 = mybir.dt.bfloat16
    i16 = mybir.dt.int16
    NWL, NPAIR, KL, KH = cfg.NWL, cfg.NPAIR, cfg.KL, cfg.KH
    K, K2, EPP = cfg.K, cfg.K2, cfg.EPP
    NTT, NT_PAD, SPLIT = cfg.NTT, cfg.NT_PAD, cfg.SPLIT
    TPH = cfg.TPH

    nc = bacc.Bacc("TRN2", target_bir_lowering=False, debug=False,
                   num_devices=NCORES, num_swdge_queues=4,
                   dynamic_dma_scratch_size=131072)
    xT = nc.dram_tensor("xT", [D_IN, NT_PAD], bf16, kind="ExternalInput").ap()
    WallT = nc.dram_tensor("WallT", [D_IN, TW], bf16,
                           kind="ExternalInput").ap()
    vT8 = nc.dram_tensor("vT8", [2 * ED_DIM, 2 * H_HEADS], bf16,
                         kind="ExternalInput").ap()
    eaT2 = nc.dram_tensor("eaT2", [2 * ED_DIM, NPAIR * K2 * P // 2], bf16,
                          kind="ExternalInput").ap()
    dstloc = nc.dram_tensor("dstloc", [P, NPAIR * K2], bf16,
                            kind="ExternalInput").ap()
    StA = nc.dram_tensor("StA", [P, NPAIR * EPP], bf16,
                         kind="ExternalInput").ap()
    invcnt = nc.dram_tensor("invcnt", [P, NWL], f32, kind="ExternalInput").ap()
    glo16 = nc.dram_tensor("glo16", [P, NPAIR * 2 * KL * 8], i16,
                           kind="ExternalInput").ap()
    ghi16 = nc.dram_tensor("ghi16", [P, NPAIR * 2 * KH * 8], i16,
                           kind="ExternalInput").ap()
    selfbase = nc.dram_tensor("selfbase", [1, 1], mybir.dt.uint32,
                              kind="ExternalInput").ap()
    out = nc.dram_tensor("out", [NWL * P, HC], f32, kind="ExternalOutput").ap()
    tableA = nc.dram_tensor("tableA", [NT_PAD + 4 * NWL, TW], bf16).ap()
    tabL3 = tableA[0:SPLIT, :].rearrange("(s t) u -> s t u", t=TPH)
    tabH3 = tableA[SPLIT:NT_PAD, :].rearrange("(s t) u -> s t u", t=TPH)

    with tile.TileContext(nc) as tc, ExitStack() as ctx:
        cpool = ctx.enter_context(tc.tile_pool(name="const", bufs=1))
        xpool = ctx.enter_context(tc.tile_pool(name="xload", bufs=2))
        tabpool = ctx.enter_context(tc.tile_pool(name="tab", bufs=2))
        eapool = ctx.enter_context(tc.tile_pool(name="ea", bufs=2))
        gpool = ctx.enter_context(tc.tile_pool(name="gather", bufs=3))
        ipool = ctx.enter_context(tc.tile_pool(name="idx", bufs=3))
        stpool = ctx.enter_context(tc.tile_pool(name="sT", bufs=1))
        spool = ctx.enter_context(tc.tile_pool(name="oneh", bufs=1))
        mpool = ctx.enter_context(tc.tile_pool(name="msg", bufs=2))
        wpool = ctx.enter_context(tc.tile_pool(name="work", bufs=3))
        opool = ctx.enter_context(tc.tile_pool(name="outw", bufs=3))
        pst = ctx.enter_context(tc.tile_pool(name="ps_t", bufs=2, space="PSUM"))
        pse = ctx.enter_context(tc.tile_pool(name="ps_e", bufs=2, space="PSUM"))
        psa = ctx.enter_context(tc.tile_pool(name="ps_a", bufs=2, space="PSUM"))
        psad = ctx.enter_context(tc.tile_pool(name="ps_ad", bufs=2,
                                              space="PSUM"))

        # ---- constants ----
        WallT_sb = cpool.tile([P, TW], bf16)
        nc.sync.dma_start(WallT_sb[:], WallT[:])
        vT8_sb = cpool.tile([2 * ED_DIM, 2 * H_HEADS], bf16)
        nc.sync.dma_start(vT8_sb[:], vT8[:])
        iota1 = cpool.tile([P, P], bf16)  # value = col index
        nc.gpsimd.iota(iota1[:], pattern=[[1, P]], base=0,
                       channel_multiplier=0,
                       allow_small_or_imprecise_dtypes=True)
        dstloc_sb = cpool.tile([P, NPAIR * K2], bf16)
        nc.sync.dma_start(dstloc_sb[:], dstloc[:])
        invcnt_sb = cpool.tile([P, NWL], f32)
        nc.sync.dma_start(invcnt_sb[:], invcnt[:])

        # ---- phase T: node table = [xh | a_src | a_dst | 0 pad] ----
        # low half (windows < TPH) first so low-table gathers start early
        XB = 8
        assert TPH % XB == 0
        for g in range(NTT // XB):
            t0 = g * XB
            tab3h = tabL3 if t0 < TPH else tabH3
            th0 = t0 if t0 < TPH else t0 - TPH
            xt = xpool.tile([P, XB * P], bf16, tag="xt")
            nc.sync.dma_start(xt[:], xT[:, t0 * P:(t0 + XB) * P])
            tab = tabpool.tile([P, XB * TW], bf16, tag="tab")
            for t4 in range(0, XB, 4):
                ps = pst.tile([P, 4 * TW], f32)
                for t in range(t4, t4 + 4):
                    nc.tensor.matmul(
                        out=ps[:, (t - t4) * TW:(t - t4 + 1) * TW],
                        lhsT=xt[:, t * P:(t + 1) * P],
                        rhs=WallT_sb[:], start=True, stop=True)
                nc.vector.tensor_copy(
                    tab[:, t4 * TW:(t4 + 4) * TW], ps[:])
            nc.scalar.dma_start(
                out=tab3h[:, th0:th0 + XB, :],
                in_=tab[:].rearrange("p (t u) -> p t u", u=TW))

        # ---- own node rows, straight into SBUF (per-core row base) ----
        selfall = cpool.tile([P, NWL * (HC + 8)], bf16)
        sreg = nc.sync.alloc_register("selfstart")
        nc.sync.reg_load(sreg, selfbase[0:1, 0:1])
        sstart = nc.sync.snap(sreg, donate=True, min_val=0,
                              max_val=SPLIT + (NCORES // 2 - 1) * NWL)
        nc.sync.dma_start(
            out=selfall[:].rearrange("p (w u) -> p w u", u=HC + 8),
            in_=tableA[bass.ds(sstart, P * TPH), 0:HC + 8]
            .rearrange("(s t) u -> s t u", t=TPH)[:, 0:NWL, :])

        # ---- phase B: per-pair attention softmax + aggregation ----
        # block j of a pair belongs to window parity blk_win[j]:
        blk_win = [0] * KL + [1] * KL + [0] * KH + [1] * KH
        win_blocks = [[j for j in range(K2) if blk_win[j] == e]
                      for e in (0, 1)]
        for v in range(NPAIR):
            gi = ipool.tile([P, 2 * (KL + KH) * 8], i16, tag="gi")
            nc.sync.dma_start(gi[:, :2 * KL * 8],
                              glo16[:, v * 2 * KL * 8:(v + 1) * 2 * KL * 8])
            nc.sync.dma_start(gi[:, 2 * KL * 8:],
                              ghi16[:, v * 2 * KH * 8:(v + 1) * 2 * KH * 8])
            G = gpool.tile([P, K2 * TW], bf16, tag="G")
            Gv = G[:].rearrange("p (k u) -> p k u", u=TW)
            nc.gpsimd.dma_gather(
                out_ap=Gv[:, 0:2 * KL, :], in_ap=tableA[0:SPLIT, :],
                idxs_ap=gi[:, 0:2 * KL * 8],
                num_idxs=2 * KL * P, num_idxs_reg=2 * KL * P, elem_size=TW,
                single_packet=False, queue_num=(2 * v) % 4)
            nc.gpsimd.dma_gather(
                out_ap=Gv[:, 2 * KL:K2, :], in_ap=tableA[SPLIT:NT_PAD, :],
                idxs_ap=gi[:, 2 * KL * 8:2 * (KL + KH) * 8],
                num_idxs=2 * KH * P, num_idxs_reg=2 * KH * P, elem_size=TW,
                single_packet=False, queue_num=(2 * v + 1) % 4)

            # transposed one-hot (lhsT of a_dst expansion): shipped from
            # host in window-major block order; one-hot S built per window
            ps_adst = psad.tile([P, K2 * UH], f32)
            Sw = []
            for e in (0, 1):
                w = 2 * v + e
                St = stpool.tile([P, K * P], bf16, tag=f"St{e}")
                nc.sync.dma_start(
                    St[:], StA[:, (2 * v + e) * K * P:(2 * v + e + 1) * K * P])
                S = spool.tile([P, K * P], bf16, tag=f"S{e}")
                nc.vector.tensor_tensor(
                    out=S[:].rearrange("p (k u) -> p k u", u=P),
                    in0=iota1[:].unsqueeze(1).broadcast_to([P, K, P]),
                    in1=dstloc_sb[:, (2 * v + e) * K:(2 * v + e + 1) * K]
                    .unsqueeze(2).broadcast_to([P, K, P]),
                    op=mybir.AluOpType.is_equal)
                Sw.append(S)
                for i, j in enumerate(win_blocks[e]):
                    nc.tensor.matmul(
                        out=ps_adst[:, j * UH:(j + 1) * UH],
                        lhsT=St[:, i * P:(i + 1) * P],
                        rhs=selfall[:, w * (HC + 8) + HC + 4:
                                    w * (HC + 8) + HC + 8],
                        start=True, stop=True)

            # a_edge0 = edge_attr @ v.T for this pair (2 blocks per matmul)
            ea_ch = eapool.tile([2 * ED_DIM, K * P], bf16, tag="ea")
            nc.sync.dma_start(ea_ch[:], eaT2[:, v * K * P:(v + 1) * K * P])
            ps_e = pse.tile([P, K2 * UH], f32)
            for jj in range(K):
                nc.tensor.matmul(
                    out=ps_e[:, jj * 8:(jj + 1) * 8],
                    lhsT=ea_ch[:, jj * P:(jj + 1) * P],
                    rhs=vT8_sb[:], start=True, stop=True)

            # alpha = a_src(src) + a_dst(dst) + a_edge
            aw = wpool.tile([P, K2 * UH], f32, tag="aw")
            aw3 = aw[:].rearrange("p (k u) -> p k u", u=UH)
            nc.vector.tensor_tensor(
                out=aw3, in0=Gv[:, :, HC:HC + UH],
                in1=ps_adst[:].rearrange("p (k u) -> p k u", u=UH),
                op=mybir.AluOpType.add)
            nc.vector.tensor_tensor(
                out=aw[:], in0=aw[:], in1=ps_e[:], op=mybir.AluOpType.add)
            # lrelu(x) = slope*x + relu((1-slope)*x), then exp
            lrl = wpool.tile([P, K2 * UH], f32, tag="lrl")
            nc.scalar.activation(lrl[:], aw[:],
                                 mybir.ActivationFunctionType.Relu,
                                 scale=1.0 - NEG_SLOPE)
            nc.vector.scalar_tensor_tensor(
                out=lrl[:], in0=aw[:], scalar=NEG_SLOPE, in1=lrl[:],
                op0=mybir.AluOpType.mult, op1=mybir.AluOpType.add)

            # M = [expal * xh | expal | a_edge0] per block
            M = mpool.tile([P, K2 * MW], bf16, tag="M")
            M3 = M[:].rearrange("p (k u) -> p k u", u=MW)
            nc.scalar.activation(M3[:, :, HC:HC + UH],
                                 lrl[:].rearrange("p (k u) -> p k u", u=UH),
                                 mybir.ActivationFunctionType.Exp)
            nc.vector.tensor_copy(
                M3[:, :, HC + 4:HC + 8],
                ps_e[:].rearrange("p (k u) -> p k u", u=UH))
            expal_b = (M3[:, :, HC:HC + UH].unsqueeze(3)
                       .broadcast_to([P, K2, UH, C_OUT]))
            nc.vector.tensor_tensor(
                out=M3[:, :, 0:HC].rearrange("p k (h c) -> p k h c", c=C_OUT),
                in0=Gv[:, :, 0:HC].rearrange("p k (h c) -> p k h c", c=C_OUT),
                in1=expal_b, op=mybir.AluOpType.mult)

            # segment sums: one matmul per block, accumulated in PSUM;
            # both windows share one PSUM tile (disjoint column ranges)
            ps_agg = psa.tile([P, 2 * MW], f32)
            for e in (0, 1):
                blocks = win_blocks[e]
                for i, j in enumerate(blocks):
                    nc.tensor.matmul(
                        out=ps_agg[:, e * MW:(e + 1) * MW],
                        lhsT=Sw[e][:, i * P:(i + 1) * P],
                        rhs=M[:, j * MW:(j + 1) * MW],
                        start=(i == 0), stop=(i == len(blocks) - 1))

            # ---- window close: self-loop term + normalization ----
            for e in (0, 1):
                w = 2 * v + e
                agg = ps_agg[:, e * MW:(e + 1) * MW]
                selfr = selfall[:, w * (HC + 8):(w + 1) * (HC + 8)]
                lae = wpool.tile([P, 4], f32, tag=f"lae{e}")
                nc.vector.tensor_scalar(
                    out=lae[:], in0=agg[:, HC + 4:HC + 8],
                    scalar1=invcnt_sb[:, w:w + 1],
                    scalar2=None, op0=mybir.AluOpType.mult)
                asf = wpool.tile([P, 4], f32, tag=f"asf{e}")
                nc.vector.tensor_tensor(
                    out=asf[:], in0=selfr[:, HC:HC + 4],
                    in1=selfr[:, HC + 4:HC + 8], op=mybir.AluOpType.add)
                nc.vector.tensor_tensor(out=asf[:], in0=asf[:], in1=lae[:],
                                        op=mybir.AluOpType.add)
                es = wpool.tile([P, 4], f32, tag=f"es{e}")
                nc.scalar.activation(es[:], asf[:],
                                     mybir.ActivationFunctionType.Relu,
                                     scale=1.0 - NEG_SLOPE)
                nc.vector.scalar_tensor_tensor(
                    out=es[:], in0=asf[:], scalar=NEG_SLOPE, in1=es[:],
                    op0=mybir.AluOpType.mult, op1=mybir.AluOpType.add)
                nc.scalar.activation(es[:], es[:],
                                     mybir.ActivationFunctionType.Exp)
                # den = exp(alpha_self) + 1e-30 + sum_edges exp(alpha)
                den = wpool.tile([P, 4], f32, tag=f"den{e}")
                nc.vector.scalar_tensor_tensor(
                    out=den[:], in0=es[:], scalar=1e-30,
                    in1=agg[:, HC:HC + 4],
                    op0=mybir.AluOpType.add, op1=mybir.AluOpType.add)
                rec = wpool.tile([P, 4], f32, tag=f"rec{e}")
                nc.vector.reciprocal(rec[:], den[:])
                ot = opool.tile([P, HC], f32, tag=f"ot{e}")
                es_b = es[:].unsqueeze(2).broadcast_to([P, 4, C_OUT])
                nc.vector.tensor_tensor(
                    out=ot[:].rearrange("p (h c) -> p h c", c=C_OUT),
                    in0=selfr[:, 0:HC].rearrange("p (h c) -> p h c", c=C_OUT),
                    in1=es_b, op=mybir.AluOpType.mult)
                nc.vector.tensor_tensor(out=ot[:], in0=ot[:],
                                        in1=agg[:, 0:HC],
                                        op=mybir.AluOpType.add)
                rec_b = rec[:].unsqueeze(2).broadcast_to([P, 4, C_OUT])
                nc.vector.tensor_tensor(
                    out=ot[:].rearrange("p (h c) -> p h c", c=C_OUT),
                    in0=ot[:].rearrange("p (h c) -> p h c", c=C_OUT),
                    in1=rec_b, op=mybir.AluOpType.mult)
                nc.sync.dma_start(out[w * P:(w + 1) * P, :], ot[:])

    nc.compile()
    return nc


_NC_CACHE = {}


def _get_nc(cfg):
    k = cfg.key()
    if k not in _NC_CACHE:
        _NC_CACHE[k] = _build_nc(cfg)
    return _NC_CACHE[k]


def kernel(**inputs):
    x = np.asarray(inputs["x"], dtype=np.float32)
    ei = np.asarray(inputs["edge_index"])
    ea = np.asarray(inputs["edge_attr"], dtype=np.float32)
    W = np.asarray(inputs["W"], dtype=np.float32)
    W_edge = np.asarray(inputs["W_edge"], dtype=np.float32)
    att_src = np.asarray(inputs["att_src"], dtype=np.float32)
    att_dst = np.asarray(inputs["att_dst"], dtype=np.float32)
    att_edge = np.asarray(inputs["att_edge"], dtype=np.float32)
    bias = np.asarray(inputs["bias"], dtype=np.float32)

    src = ei[0].astype(np.int64)
    dst = ei[1].astype(np.int64)
    WallT, vT8 = _fold_weights(W, W_edge, att_src, att_dst, att_edge)

    cfg, in_maps, meta = _prep(x, src, dst, ea, WallT, vT8)
    nc = _get_nc(cfg)

    from concourse.bass_utils import run_bass_kernel_spmd
    res = run_bass_kernel_spmd(nc, in_maps, core_ids=list(range(NCORES)),
                               trace=TRACE)
    if TRACE:
        global LAST_RESULT
        LAST_RESULT = res

    out_ws = np.concatenate([res.results[c]["out"] for c in range(NCORES)],
                            axis=0)  # window-space [n_windows*P, HC]
    out = out_ws[meta["winpos"]]
    return (out + bias[None, :]).astype(np.float32)


# revision 20
# speedup vs baseline: 1.6982x; 1.1926x over previous
"""GAT message-passing kernel for Trainium2, 8 NeuronCores, dst-partitioned.

v3 (bf16, paired windows, shipped transposed one-hot, p-major table):
 - Fold attention vectors into the linear weights on host (tiny matmuls):
   a_src = x @ u_src.T, a_dst = x @ u_dst.T, a_edge = edge_attr @ v.T.
 - Softmax over incoming edges is computed WITHOUT max-subtraction (logits
   are bounded so exp cannot overflow; softmax is shift-invariant) so only
   segment-SUMS are needed, which map onto TensorE one-hot matmuls.
 - Host packs destination nodes into 128-slot windows (slot 127 of every
   window is a trash slot that absorbs padded edges), balanced by in-degree
   (LPT). Windows are processed in PAIRS: one gather instruction per table
   half per pair (~2300 rows each) amortizes the ~1us SWDGE fixed cost;
   gathers rotate across the 4 SWDGE queues (4 Q7 core pairs + descriptor
   rings), which overlaps descriptor generation with DMA drain.
 - Everything on device is bf16 (tolerance 2e-2; bf16 adds ~0.5%):
   PE matmuls run 4x faster and gather rows are 256B.
 - Node table rows are PARTITION-MAJOR: row(node) = slot*NTT + window, so
   phase T writes the table with one contiguous 2KB descriptor per
   partition per chunk instead of 256B/row descriptors, and a core's own
   rows [slot, ds(core*NWL+w)] load straight into SBUF with one DMA.
   Low/high table split (int16 gather indices) is slot<64 vs slot>=64.
 - Per pair, the one-hot S [edge, node] is built in ONE DVE op (iota vs
   dstloc broadcast); the transposed one-hot St [node, edge] (lhsT of the
   a_dst expansion matmul) is SHIPPED from host — it is pure index data,
   and DVE compare/broadcast ops run at ~1/4 copy speed, so a DMA is
   cheaper than rebuilding or PE-transposing (which needs a PSUM round
   trip of the same size).
 - Self-loops (PyG GATConv: loop edge_attr = per-dst mean of incoming
   edge_attr) fold in at window close from the unweighted aedge segment
   sum that rides the aggregation matmul.
"""

import math

import numpy as np

NCORES = 8
D_IN = 128
H_HEADS = 4
C_OUT = 16
HC = H_HEADS * C_OUT  # 64
ED_DIM = 64
NEG_SLOPE = 0.2
TW = 128             # table row width (bf16) -> 256B rows for dma_gather
UH = H_HEADS
MW = HC + 8          # M columns per block: [expal*xh | expal | a_edge0]

P = 128  # partitions / window slot count (127 real nodes + trash slot)

TRACE = False       # set by test harness to capture an NTFF profile
LAST_RESULT = None  # BassKernelResults of the last traced run


class _Cfg:
    def __init__(self, nwl, kl, kh, ncores):
        self.NWL = nwl            # windows per core (even)
        self.NPAIR = nwl // 2
        self.KL = kl              # low-half edge blocks per window
        self.KH = kh              # high-half edge blocks per window
        self.K = kl + kh          # 128-edge blocks per window
        self.K2 = 2 * self.K      # blocks per window pair
        self.EPP = self.K2 * P    # edge slots per window pair
        self.NTT = ncores * nwl   # global window count
        self.TPH = self.NTT // 2  # windows per table half (cores 0..3 = low)
        self.NT_PAD = self.NTT * P
        self.NSLOTS = self.NT_PAD
        self.SPLIT = P * self.TPH  # low-table rows
        self.ECB = nwl * self.K   # edge blocks per core

    def key(self):
        return (self.NWL, self.KL, self.KH, self.NTT)


def _fold_weights(W, W_edge, att_src, att_dst, att_edge):
    H, C = att_src.shape
    D = W.shape[1]
    ED = W_edge.shape[1]
    u_src = np.einsum("hc,hcd->hd", att_src, W.reshape(H, C, D))
    u_dst = np.einsum("hc,hcd->hd", att_dst, W.reshape(H, C, D))
    v = np.einsum("hc,hcd->hd", att_edge, W_edge.reshape(H, C, ED))
    # WallT columns = [W.T | u_src.T | u_dst.T | zero pad to TW]
    WallT = np.zeros((D, TW), np.float32)
    WallT[:, :HC] = W.T
    WallT[:, HC:HC + H] = u_src.T
    WallT[:, HC + H:HC + 2 * H] = u_dst.T
    # vT8: rows 0:ED -> [v.T | 0], rows ED:2ED -> [0 | v.T]  (paired matmul)
    vT8 = np.zeros((2 * ED, 2 * H), np.float32)
    vT8[:ED, :H] = v.T
    vT8[ED:, H:] = v.T
    return WallT, vT8


def _partition_nodes(dst, n_nodes, n_windows):
    """LPT-pack nodes into n_windows bins of <=127 nodes each (slot 127 is
    the trash slot), balancing in-degree sums."""
    import heapq

    cap = P - 1
    deg = np.bincount(dst, minlength=n_nodes).astype(np.int64)
    order = np.argsort(-deg, kind="stable")
    heap = [(0, w) for w in range(n_windows)]
    heapq.heapify(heap)
    win_of = np.empty(n_nodes, np.int32)
    slot_of = np.empty(n_nodes, np.int32)
    nodes_in = np.zeros(n_windows, np.int32)
    edges_in = np.zeros(n_windows, np.int64)
    for n in order:
        while True:
            e, w = heapq.heappop(heap)
            if nodes_in[w] < cap:
                break  # full windows are dropped from the heap for good
        win_of[n] = w
        slot_of[n] = nodes_in[w]
        nodes_in[w] += 1
        edges_in[w] += deg[n]
        if nodes_in[w] < cap:
            heapq.heappush(heap, (int(edges_in[w]), w))
    return win_of, slot_of


def _wrap16(idx, num):
    """int16 index array -> dma_gather layout: item i lives at partition
    i%16, col i//16; replicated down the remaining 112 partitions."""
    a = idx.astype(np.int16).reshape(num // 16, 16).T  # [16, num//16]
    return np.ascontiguousarray(np.tile(a, (8, 1)))


def _prep(x, src, dst, edge_attr, WallT, vT8):
    """Build per-core input maps + meta for unsharding."""
    import ml_dtypes
    bf = ml_dtypes.bfloat16

    n = x.shape[0]
    nwl = math.ceil(n / ((P - 1) * NCORES))
    if nwl % 2:
        nwl += 1  # windows are processed in pairs
    n_windows = NCORES * nwl

    win_of, slot_of = _partition_nodes(dst, n, n_windows)
    R_TRASH = P - 1

    winpos = win_of.astype(np.int64) * P + slot_of
    ntt = n_windows
    tph = ntt // 2
    # table row: half = (window >= tph); row = half*SPLIT + slot*tph + t_loc
    halfv = (win_of >= tph).astype(np.int64)
    row_of = (halfv * (P * tph) + slot_of.astype(np.int64) * tph
              + win_of - halfv * tph)
    split = P * tph
    assert split <= 32768 and ntt * P - split <= 32767

    ewin = win_of[dst]
    srow = row_of[src]
    is_low = (win_of[src] < tph)

    # fixed per-window low/high block counts across all cores (SPMD)
    nlow = np.bincount(ewin[is_low], minlength=n_windows)
    nhigh = np.bincount(ewin[~is_low], minlength=n_windows)
    kl = max(1, math.ceil(nlow.max() / P))
    kh = max(1, math.ceil(nhigh.max() / P))
    cfg = _Cfg(nwl, kl, kh, NCORES)
    K2, EPP = cfg.K2, cfg.EPP
    npair_g = n_windows // 2

    # ---- place edges pair-major: [low(2v) | low(2v+1) | hi(2v) | hi(2v+1)],
    #      each region padded to a block multiple ----
    pairg = ewin.astype(np.int64) // 2
    parity = ewin.astype(np.int64) % 2
    half = (~is_low).astype(np.int64)
    grp = pairg * 4 + half * 2 + parity
    order_e = np.argsort(grp, kind="stable")
    grp_s = grp[order_e]
    counts = np.bincount(grp_s, minlength=4 * npair_g)
    offs = np.zeros(4 * npair_g + 1, np.int64)
    np.cumsum(counts, out=offs[1:])
    pos = np.arange(len(order_e), dtype=np.int64) - offs[grp_s]
    roff = np.array([0, kl * P, 2 * kl * P, (2 * kl + kh) * P], np.int64)
    q = (grp_s // 4) * EPP + roff[grp_s % 4] + pos

    Q = npair_g * EPP
    lowmask_q = (np.arange(Q) % EPP) < 2 * kl * P
    gsrc_q = np.zeros(Q, np.int64)  # pads gather row 0 (harmless: trash dst)
    dstloc_q = np.full(Q, R_TRASH, np.int16)
    gsrc_q[q] = srow[order_e]
    dstloc_q[q] = slot_of[dst[order_e]].astype(np.int16)

    ea_q = np.zeros((Q, ED_DIM), np.float32)
    ea_q[q] = edge_attr[order_e]

    # node features in winpos (window-major) order: phase-T block t is
    # global window t, partition = slot
    x_ws = np.zeros((cfg.NT_PAD, D_IN), np.float32)
    x_ws[winpos] = x
    xT = np.ascontiguousarray(x_ws.T.astype(bf))  # [D_IN, NT_PAD]

    invcnt_ws = np.ones(n_windows * P, np.float32)
    cnt = np.bincount(dst, minlength=n).astype(np.float32)
    invcnt_ws[winpos] = 1.0 / np.maximum(cnt, 1.0)

    glow_q = np.where(lowmask_q, gsrc_q, 0)
    ghigh_q = np.where(lowmask_q, 0, np.maximum(gsrc_q - split, 0))
    assert glow_q.max() < split and ghigh_q.max() < ntt * P - split

    # window-major block order within a pair: [w0 low, w0 high, w1 low, w1 hi]
    kl2, kh2, K2c = kl, kh, K2
    blk_order = (list(range(0, kl2)) + list(range(2 * kl2, 2 * kl2 + kh2))
                 + list(range(kl2, 2 * kl2))
                 + list(range(2 * kl2 + kh2, K2c)))
    in_maps = []
    npair = cfg.NPAIR
    WallT16 = WallT.astype(bf)
    vT816 = vT8.astype(bf)
    slot_ar = np.arange(P, dtype=np.int16)
    for c in range(NCORES):
        qs, qe = c * npair * EPP, (c + 1) * npair * EPP
        dq = dstloc_q[qs:qe]
        eac = ea_q[qs:qe].reshape(npair * K2 // 2, 2, P, ED_DIM)
        eaT2 = np.ascontiguousarray(
            eac.transpose(1, 3, 0, 2).reshape(2 * ED_DIM, -1)).astype(bf)
        dq_re = dq.reshape(npair, K2, P)[:, blk_order, :].reshape(-1)
        dstloc_c = np.ascontiguousarray(
            dq_re.reshape(npair * K2, P).T.astype(bf))   # [P, NPAIR*K2]
        StA = np.ascontiguousarray(
            (dq_re[None, :] == slot_ar[:, None]).astype(bf))
        lo = glow_q[qs:qe].reshape(npair, EPP)
        hi = ghigh_q[qs:qe].reshape(npair, EPP)
        glo16 = np.concatenate(
            [_wrap16(lo[v, :2 * kl * P], 2 * kl * P) for v in range(npair)],
            axis=1)
        ghi16 = np.concatenate(
            [_wrap16(hi[v, 2 * kl * P:], 2 * kh * P) for v in range(npair)],
            axis=1)
        invcnt_c = np.ascontiguousarray(
            invcnt_ws[c * nwl * P:(c + 1) * nwl * P].reshape(nwl, P).T
            .astype(np.float32))
        selfbase = np.array([[(c // (NCORES // 2)) * split
                              + (c % (NCORES // 2)) * nwl]], np.uint32)
        in_maps.append(dict(
            xT=xT, eaT2=eaT2, dstloc=dstloc_c, StA=StA,
            invcnt=invcnt_c, glo16=glo16, ghi16=ghi16,
            WallT=WallT16, vT8=vT816, selfbase=selfbase,
        ))
    meta = dict(winpos=winpos, cfg=cfg)
    return cfg, in_maps, meta


def _build_nc(cfg):
    import concourse.bass as bass
    import concourse.tile as tile
    from concourse import bacc, mybir
    from contextlib import ExitStack

    f32 = mybir.dt.float32
    bf16 = mybir.dt.bfloat16
    i16 = mybir.dt.int16
    NWL, NPAIR, KL, KH = cfg.NWL, cfg.NPAIR, cfg.KL, cfg.KH
    K, K2, EPP = cfg.K, cfg.K2, cfg.EPP
    NTT, NT_PAD, SPLIT = cfg.NTT, cfg.NT_PAD, cfg.SPLIT
    TPH = cfg.TPH

    nc = bacc.Bacc("TRN2", target_bir_lowering=False, debug=False,
                   num_devices=NCORES, num_swdge_queues=4,
                   dynamic_dma_scratch_size=65536)
    xT = nc.dram_tensor("xT", [D_IN, NT_PAD], bf16, kind="ExternalInput").ap()
    WallT = nc.dram_tensor("WallT", [D_IN, TW], bf16,
                           kind="ExternalInput").ap()
    vT8 = nc.dram_tensor("vT8", [2 * ED_DIM, 2 * H_HEADS], bf16,
                         kind="ExternalInput").ap()
    eaT2 = nc.dram_tensor("eaT2", [2 * ED_DIM, NPAIR * K2 * P // 2], bf16,
                          kind="ExternalInput").ap()
    dstloc = nc.dram_tensor("dstloc", [P, NPAIR * K2], bf16,
                            kind="ExternalInput").ap()
    StA = nc.dram_tensor("StA", [P, NPAIR * EPP], bf16,
                         kind="ExternalInput").ap()
    invcnt = nc.dram_tensor("invcnt", [P, NWL], f32, kind="ExternalInput").ap()
    glo16 = nc.dram_tensor("glo16", [P, NPAIR * 2 * KL * 8], i16,
                           kind="ExternalInput").ap()
    ghi16 = nc.dram_tensor("ghi16", [P, NPAIR * 2 * KH * 8], i16,
                           kind="ExternalInput").ap()
    selfbase = nc.dram_tensor("selfbase", [1, 1], mybir.dt.uint32,
                              kind="ExternalInput").ap()
    out = nc.dram_tensor("out", [NWL * P, HC], f32, kind="ExternalOutput").ap()
    tableA = nc.dram_tensor("tableA", [NT_PAD + 4 * NWL, TW], bf16).ap()
    tabL3 = tableA[0:SPLIT, :].rearrange("(s t) u -> s t u", t=TPH)
    tabH3 = tableA[SPLIT:NT_PAD, :].rearrange("(s t) u -> s t u", t=TPH)

    with tile.TileContext(nc) as tc, ExitStack() as ctx:
        cpool = ctx.enter_context(tc.tile_pool(name="const", bufs=1))
        xpool = ctx.enter_context(tc.tile_pool(name="xload", bufs=2))
        tabpool = ctx.enter_context(tc.tile_pool(name="tab", bufs=2))
        eapool = ctx.enter_context(tc.tile_pool(name="ea", bufs=2))
        gpool = ctx.enter_context(tc.tile_pool(name="gather", bufs=3))
        ipool = ctx.enter_context(tc.tile_pool(name="idx", bufs=3))
        stpool = ctx.enter_context(tc.tile_pool(name="sT", bufs=1))
        spool = ctx.enter_context(tc.tile_pool(name="oneh", bufs=2))
        mpool = ctx.enter_context(tc.tile_pool(name="msg", bufs=2))
        wpool = ctx.enter_context(tc.tile_pool(name="work", bufs=3))
        opool = ctx.enter_context(tc.tile_pool(name="outw", bufs=3))
        pst = ctx.enter_context(tc.tile_pool(name="ps_t", bufs=2, space="PSUM"))
        pse = ctx.enter_context(tc.tile_pool(name="ps_e", bufs=2, space="PSUM"))
        psa = ctx.enter_context(tc.tile_pool(name="ps_a", bufs=2, space="PSUM"))
        psad = ctx.enter_context(tc.tile_pool(name="ps_ad", bufs=2,
                                              space="PSUM"))

        # ---- constants ----
        WallT_sb = cpool.tile([P, TW], bf16)
        nc.sync.dma_start(WallT_sb[:], WallT[:])
        vT8_sb = cpool.tile([2 * ED_DIM, 2 * H_HEADS], bf16)
        nc.sync.dma_start(vT8_sb[:], vT8[:])
        iota1 = cpool.tile([P, P], bf16)  # value = col index
        nc.gpsimd.iota(iota1[:], pattern=[[1, P]], base=0,
                       channel_multiplier=0,
                       allow_small_or_imprecise_dtypes=True)
        dstloc_sb = cpool.tile([P, NPAIR * K2], bf16)
        nc.sync.dma_start(dstloc_sb[:], dstloc[:])
        invcnt_sb = cpool.tile([P, NWL], f32)
        nc.sync.dma_start(invcnt_sb[:], invcnt[:])

        # ---- phase T: node table = [xh | a_src | a_dst | 0 pad] ----
        # low half (windows < TPH) first so low-table gathers start early
        XB = 8
        assert TPH % XB == 0
        for g in range(NTT // XB):
            t0 = g * XB
            tab3h = tabL3 if t0 < TPH else tabH3
            th0 = t0 if t0 < TPH else t0 - TPH
            xt = xpool.tile([P, XB * P], bf16, tag="xt")
            nc.sync.dma_start(xt[:], xT[:, t0 * P:(t0 + XB) * P])
            tab = tabpool.tile([P, XB * TW], bf16, tag="tab")
            for t4 in range(0, XB, 4):
                ps = pst.tile([P, 4 * TW], f32)
                for t in range(t4, t4 + 4):
                    nc.tensor.matmul(
                        out=ps[:, (t - t4) * TW:(t - t4 + 1) * TW],
                        lhsT=xt[:, t * P:(t + 1) * P],
                        rhs=WallT_sb[:], start=True, stop=True)
                nc.vector.tensor_copy(
                    tab[:, t4 * TW:(t4 + 4) * TW], ps[:])
            nc.scalar.dma_start(
                out=tab3h[:, th0:th0 + XB, :],
                in_=tab[:].rearrange("p (t u) -> p t u", u=TW))

        # ---- own node rows, straight into SBUF (per-core row base) ----
        selfall = cpool.tile([P, NWL * (HC + 8)], bf16)
        sreg = nc.sync.alloc_register("selfstart")
        nc.sync.reg_load(sreg, selfbase[0:1, 0:1])
        sstart = nc.sync.snap(sreg, donate=True, min_val=0,
                              max_val=SPLIT + (NCORES // 2 - 1) * NWL)
        nc.sync.dma_start(
            out=selfall[:].rearrange("p (w u) -> p w u", u=HC + 8),
            in_=tableA[bass.ds(sstart, P * TPH), 0:HC + 8]
            .rearrange("(s t) u -> s t u", t=TPH)[:, 0:NWL, :])

        # ---- phase B: per-pair attention softmax + aggregation ----
        # block j of a pair belongs to window parity blk_win[j]:
        blk_win = [0] * KL + [1] * KL + [0] * KH + [1] * KH
        win_blocks = [[j for j in range(K2) if blk_win[j] == e]
                      for e in (0, 1)]
        for v in range(NPAIR):
            gi = ipool.tile([P, 2 * (KL + KH) * 8], i16, tag="gi")
            nc.sync.dma_start(gi[:, :2 * KL * 8],
                              glo16[:, v * 2 * KL * 8:(v + 1) * 2 * KL * 8])
            nc.sync.dma_start(gi[:, 2 * KL * 8:],
                              ghi16[:, v * 2 * KH * 8:(v + 1) * 2 * KH * 8])
            G = gpool.tile([P, K2 * TW], bf16, tag="G")
            Gv = G[:].rearrange("p (k u) -> p k u", u=TW)
            nc.gpsimd.dma_gather(
                out_ap=Gv[:, 0:2 * KL, :], in_ap=tableA[0:SPLIT, :],
                idxs_ap=gi[:, 0:2 * KL * 8],
                num_idxs=2 * KL * P, num_idxs_reg=2 * KL * P, elem_size=TW,
                single_packet=False, queue_num=(2 * v) % 4)
            nc.gpsimd.dma_gather(
                out_ap=Gv[:, 2 * KL:K2, :], in_ap=tableA[SPLIT:NT_PAD, :],
                idxs_ap=gi[:, 2 * KL * 8:2 * (KL + KH) * 8],
                num_idxs=2 * KH * P, num_idxs_reg=2 * KH * P, elem_size=TW,
                single_packet=False, queue_num=(2 * v + 1) % 4)

            # transposed one-hot (lhsT of a_dst expansion): shipped from
            # host in window-major block order; one-hot S built per window
            ps_adst = psad.tile([P, K2 * UH], f32)
            Sw = []
            for e in (0, 1):
                w = 2 * v + e
                St = stpool.tile([P, K * P], bf16, tag=f"St{e}")
                nc.sync.dma_start(
                    St[:], StA[:, (2 * v + e) * K * P:(2 * v + e + 1) * K * P])
                S = spool.tile([P, K * P], bf16, tag=f"S{e}")
                nc.vector.tensor_tensor(
                    out=S[:].rearrange("p (k u) -> p k u", u=P),
                    in0=iota1[:].unsqueeze(1).broadcast_to([P, K, P]),
                    in1=dstloc_sb[:, (2 * v + e) * K:(2 * v + e + 1) * K]
                    .unsqueeze(2).broadcast_to([P, K, P]),
                    op=mybir.AluOpType.is_equal)
                Sw.append(S)
                for i, j in enumerate(win_blocks[e]):
                    nc.tensor.matmul(
                        out=ps_adst[:, j * UH:(j + 1) * UH],
                        lhsT=St[:, i * P:(i + 1) * P],
                        rhs=selfall[:, w * (HC + 8) + HC + 4:
                                    w * (HC + 8) + HC + 8],
                        start=True, stop=True)

            # a_edge0 = edge_attr @ v.T for this pair (2 blocks per matmul)
            ea_ch = eapool.tile([2 * ED_DIM, K * P], bf16, tag="ea")
            nc.sync.dma_start(ea_ch[:], eaT2[:, v * K * P:(v + 1) * K * P])
            ps_e = pse.tile([P, K2 * UH], f32)
            for jj in range(K):
                nc.tensor.matmul(
                    out=ps_e[:, jj * 8:(jj + 1) * 8],
                    lhsT=ea_ch[:, jj * P:(jj + 1) * P],
                    rhs=vT8_sb[:], start=True, stop=True)

            # alpha = a_src(src) + a_dst(dst) + a_edge
            aw = wpool.tile([P, K2 * UH], f32, tag="aw")
            aw3 = aw[:].rearrange("p (k u) -> p k u", u=UH)
            nc.vector.tensor_tensor(
                out=aw3, in0=Gv[:, :, HC:HC + UH],
                in1=ps_adst[:].rearrange("p (k u) -> p k u", u=UH),
                op=mybir.AluOpType.add)
            nc.vector.tensor_tensor(
                out=aw[:], in0=aw[:], in1=ps_e[:], op=mybir.AluOpType.add)
            # lrelu(x) = slope*x + relu((1-slope)*x), then exp
            lrl = wpool.tile([P, K2 * UH], f32, tag="lrl")
            nc.scalar.activation(lrl[:], aw[:],
                                 mybir.ActivationFunctionType.Relu,
                                 scale=1.0 - NEG_SLOPE)
            nc.vector.scalar_tensor_tensor(
                out=lrl[:], in0=aw[:], scalar=NEG_SLOPE, in1=lrl[:],
                op0=mybir.AluOpType.mult, op1=mybir.AluOpType.add)

            # M = [expal * xh | expal | a_edge0] per block
            M = mpool.tile([P, K2 * MW], bf16, tag="M")
            M3 = M[:].rearrange("p (k u) -> p k u", u=MW)
            nc.scalar.activation(M3[:, :, HC:HC + UH],
                                 lrl[:].rearrange("p (k u) -> p k u", u=UH),
                                 mybir.ActivationFunctionType.Exp)
            nc.vector.tensor_copy(
                M3[:, :, HC + 4:HC + 8],
                ps_e[:].rearrange("p (k u) -> p k u", u=UH))
            expal_b = (M3[:, :, HC:HC + UH].unsqueeze(3)
                       .broadcast_to([P, K2, UH, C_OUT]))
            nc.vector.tensor_tensor(
                out=M3[:, :, 0:HC].rearrange("p k (h c) -> p k h c", c=C_OUT),
                in0=Gv[:, :, 0:HC].rearrange("p k (h c) -> p k h c", c=C_OUT),
                in1=expal_b, op=mybir.AluOpType.mult)

            # segment sums: one matmul per block, accumulated in PSUM;
            # both windows share one PSUM tile (disjoint column ranges)
            ps_agg = psa.tile([P, 2 * MW], f32)
            for e in (0, 1):
                blocks = win_blocks[e]
                for i, j in enumerate(blocks):
                    nc.tensor.matmul(
                        out=ps_agg[:, e * MW:(e + 1) * MW],
                        lhsT=Sw[e][:, i * P:(i + 1) * P],
                        rhs=M[:, j * MW:(j + 1) * MW],
                        start=(i == 0), stop=(i == len(blocks) - 1))

            # ---- window close: self-loop term + normalization ----
            for e in (0, 1):
                w = 2 * v + e
                agg = ps_agg[:, e * MW:(e + 1) * MW]
                selfr = selfall[:, w * (HC + 8):(w + 1) * (HC + 8)]
                lae = wpool.tile([P, 4], f32, tag=f"lae{e}")
                nc.vector.tensor_scalar(
                    out=lae[:], in0=agg[:, HC + 4:HC + 8],
                    scalar1=invcnt_sb[:, w:w + 1],
                    scalar2=None, op0=mybir.AluOpType.mult)
                asf = wpool.tile([P, 4], f32, tag=f"asf{e}")
                nc.vector.tensor_tensor(
                    out=asf[:], in0=selfr[:, HC:HC + 4],
                    in1=selfr[:, HC + 4:HC + 8], op=mybir.AluOpType.add)
                nc.vector.tensor_tensor(out=asf[:], in0=asf[:], in1=lae[:],
                                        op=mybir.AluOpType.add)
                es = wpool.tile([P, 4], f32, tag=f"es{e}")
                nc.scalar.activation(es[:], asf[:],
                                     mybir.ActivationFunctionType.Relu,
                                     scale=1.0 - NEG_SLOPE)
                nc.vector.scalar_tensor_tensor(
                    out=es[:], in0=asf[:], scalar=NEG_SLOPE, in1=es[:],
                    op0=mybir.AluOpType.mult, op1=mybir.AluOpType.add)
                nc.scalar.activation(es[:], es[:],
                                     mybir.ActivationFunctionType.Exp)
                # den = exp(alpha_self) + 1e-30 + sum_edges exp(alpha)
                den = wpool.tile([P, 4], f32, tag=f"den{e}")
                nc.vector.scalar_tensor_tensor(
                    out=den[:], in0=es[:], scalar=1e-30,
                    in1=agg[:, HC:HC + 4],
                    op0=mybir.AluOpType.add, op1=mybir.AluOpType.add)
                rec = wpool.tile([P, 4], f32, tag=f"rec{e}")
                nc.vector.reciprocal(rec[:], den[:])
                ot = opool.tile([P, HC], f32, tag=f"ot{e}")
                es_b = es[:].unsqueeze(2).broadcast_to([P, 4, C_OUT])
                nc.vector.tensor_tensor(
                    out=ot[:].rearrange("p (h c) -> p h c", c=C_OUT),
                    in0=selfr[:, 0:HC].rearrange("p (h c) -> p h c", c=C_OUT),
                    in1=es_b, op=mybir.AluOpType.mult)
                nc.vector.tensor_tensor(out=ot[:], in0=ot[:],
                                        in1=agg[:, 0:HC],
                                        op=mybir.AluOpType.add)
                rec_b = rec[:].unsqueeze(2).broadcast_to([P, 4, C_OUT])
                nc.vector.tensor_tensor(
                    out=ot[:].rearrange("p (h c) -> p h c", c=C_OUT),
                    in0=ot[:].rearrange("p (h c) -> p h c", c=C_OUT),
                    in1=rec_b, op=mybir.AluOpType.mult)
                nc.sync.dma_start(out[w * P:(w + 1) * P, :], ot[:])

    nc.compile()
    return nc


_NC_CACHE = {}


def _get_nc(cfg):
    k = cfg.key()
    if k not in _NC_CACHE:
        _NC_CACHE[k] = _build_nc(cfg)
    return _NC_CACHE[k]


def kernel(**inputs):
    x = np.asarray(inputs["x"], dtype=np.float32)
    ei = np.asarray(inputs["edge_index"])
    ea = np.asarray(inputs["edge_attr"], dtype=np.float32)
    W = np.asarray(inputs["W"], dtype=np.float32)
    W_edge = np.asarray(inputs["W_edge"], dtype=np.float32)
    att_src = np.asarray(inputs["att_src"], dtype=np.float32)
    att_dst = np.asarray(inputs["att_dst"], dtype=np.float32)
    att_edge = np.asarray(inputs["att_edge"], dtype=np.float32)
    bias = np.asarray(inputs["bias"], dtype=np.float32)

    src = ei[0].astype(np.int64)
    dst = ei[1].astype(np.int64)
    WallT, vT8 = _fold_weights(W, W_edge, att_src, att_dst, att_edge)

    cfg, in_maps, meta = _prep(x, src, dst, ea, WallT, vT8)
    nc = _get_nc(cfg)

    from concourse.bass_utils import run_bass_kernel_spmd
    res = run_bass_kernel_spmd(nc, in_maps, core_ids=list(range(NCORES)),
                               trace=TRACE)
    if TRACE:
        global LAST_RESULT
        LAST_RESULT = res

    out_ws = np.concatenate([res.results[c]["out"] for c in range(NCORES)],
                            axis=0)  # window-space [n_windows*P, HC]
    out = out_ws[meta["winpos"]]
    return (out + bias[None, :]).astype(np.float32)
